# revision 1
# baseline (speedup 1.0000x reference)
"""Distributed Trainium2 kernel for the AttentionBlock problem.

Sharding: data-parallel over batch (2) x sequence-parallel over query rows
(4 blocks of 512) = 8 cores. Each core computes QKV projection for its local
512 rows, RoPEs q/k, all-gathers K/V (bf16) within its batch group of 4
cores, runs full-softmax attention for its query rows over all 16 heads, and
applies the output projection. Outputs concatenate on the host - no
reduction needed.

Device layout notes:
- All matmul inputs are bf16 (host pre-casts); PSUM accumulates f32.
- w_in columns are host-permuted to [q_x1 | q_x2 | k_x1 | k_x2 | v] so the
  transposed projection (channels on partitions) yields rope-ready tiles.
- Scores are computed transposed (k on partitions, q free) so softmax's exp
  feeds the PV matmul directly; the softmax denominator comes from an
  all-ones 65th column appended to V in the PV matmul.
"""

import numpy as np
import ml_dtypes

BF16 = ml_dtypes.bfloat16
H, HD, D, B, S = 16, 64, 1024, 2, 2048
LS = 512           # local seq rows per core
NC_ = 8
ROPE_THETA = 10000.0

_COMPILED = None   # (nc, input_names)


def _build(stage=3):
    import concourse.bass as bass
    import concourse.mybir as mybir
    import concourse.tile as tile
    from concourse import bacc

    fp32 = mybir.dt.float32
    bf16 = mybir.dt.bfloat16

    nc = bacc.Bacc(
        "TRN2", target_bir_lowering=False, debug=False, num_devices=NC_
    )

    xT = nc.dram_tensor("xT", [D, LS], bf16, kind="ExternalInput")
    wqkv = nc.dram_tensor("wqkv", [D, 3 * D], bf16, kind="ExternalInput")
    wout = nc.dram_tensor("wout", [D, D], bf16, kind="ExternalInput")
    cosr = nc.dram_tensor("cosr", [128, LS], bf16, kind="ExternalInput")
    sinr = nc.dram_tensor("sinr", [128, LS], bf16, kind="ExternalInput")
    outT = nc.dram_tensor("outT", [D, LS], fp32, kind="ExternalOutput")

    Exp = mybir.ActivationFunctionType.Exp
    Copy = mybir.ActivationFunctionType.Copy

    with tile.TileContext(nc) as tc:
        dma = nc.default_dma_engine
        _keep = []  # hold tc.tile free-handles so GC can't release pools early

        def _single(*args, **kwargs):
            t, f = tc.tile(*args, **kwargs)
            _keep.append(f)
            return t

        # ---- persistent SBUF tensors ----
        xT_sb = _single([128, 8, LS], bf16, name="xT_sb")        # d-chunks
        qattn = _single([64, H * LS], bf16, name="qattn")        # per-head q^T
        attn_n = _single([128, 8, LS], bf16, name="attn_n")      # normalized attn^T
        ones64 = _single([1, 64], bf16, name="ones64")
        nc.vector.memset(ones64[:], 1.0)

        for dc in range(8):
            dma.dma_start(out=xT_sb[:, dc, :], in_=xT[128 * dc:128 * (dc + 1), :])

        # ---- internal DRAM for the collective ----
        # one packed buffer: rows 0:1024 = roped k^T, rows 1024:2048 = V
        # (natural (512, 1024) layout viewed as (1024, 512) rows)
        agin = _single([2048, LS], bf16, space="DRAM", name="agin")
        agout = _single([4 * 2048, LS], bf16, space="DRAM", name="agout")
        agin_k = agin[0:1024, :]
        agin_v = agin[1024:2048, :].rearrange("(s two) c -> s (two c)", two=2)

        def agout_k_rr(rr):
            return agout[2048 * rr:2048 * rr + 1024, :]

        def agout_v_rr(rr):
            return agout[2048 * rr + 1024:2048 * rr + 2048, :].rearrange(
                "(s two) c -> s (two c)", two=2)

        # ================= phase 1: K/V projection (+ k rope) ==============
        with (
            tc.tile_pool(name="w_pool", bufs=3) as w_pool,
            tc.tile_pool(name="ps_kq", bufs=4, space="PSUM") as ps_kq,
            tc.tile_pool(name="ps_v", bufs=2, space="PSUM") as ps_v,
            tc.tile_pool(name="trig", bufs=1) as trig,
            tc.tile_pool(name="rope_t", bufs=4) as rope_t,
            tc.tile_pool(name="rope_o", bufs=4) as rope_o,
            tc.tile_pool(name="vcp", bufs=2) as vcp,
        ):
            cos_sb = trig.tile([128, LS], bf16)
            sin_sb = trig.tile([128, LS], bf16)
            dma.dma_start(out=cos_sb[:], in_=cosr[:, :])
            dma.dma_start(out=sin_sb[:], in_=sinr[:, :])

            def proj_T(col0):
                """psum (128 c, LS) = wqkv[:, col0:col0+128].T @ x  (channels
                on partitions)."""
                ps = ps_kq.tile([128, LS], fp32, tag="pskq")
                for d8 in range(8):
                    wtile = w_pool.tile([128, 128], bf16, tag="w")
                    dma.dma_start(
                        out=wtile[:],
                        in_=wqkv[128 * d8:128 * (d8 + 1), col0:col0 + 128],
                    )
                    nc.tensor.matmul(
                        ps[:], wtile[:], xT_sb[:, d8, :],
                        start=(d8 == 0), stop=(d8 == 7),
                    )
                return ps

            def rope_pair(ps1, ps2):
                """Returns (x1', x2') bf16 sbuf tiles (128, LS)."""
                t1 = rope_t.tile([128, LS], bf16, tag="rt")
                t2 = rope_t.tile([128, LS], bf16, tag="rt")
                o1 = rope_o.tile([128, LS], bf16, tag="ro")
                o2 = rope_o.tile([128, LS], bf16, tag="ro")
                nc.vector.tensor_mul(t1[:], ps1[:], cos_sb[:])
                nc.vector.tensor_mul(t2[:], ps2[:], sin_sb[:])
                nc.vector.tensor_sub(o1[:], t1[:], t2[:])
                nc.vector.tensor_mul(t1[:], ps1[:], sin_sb[:])
                nc.vector.tensor_mul(t2[:], ps2[:], cos_sb[:])
                nc.vector.tensor_add(o2[:], t1[:], t2[:])
                return o1, o2

            # K projection + rope -> agin_k
            for j in range(4):
                ps1 = proj_T(1024 + 128 * j)          # k_x1 chunk j
                ps2 = proj_T(1536 + 128 * j)          # k_x2 chunk j
                o1, o2 = rope_pair(ps1, ps2)
                dma.dma_start(out=agin_k[128 * j:128 * (j + 1), :], in_=o1[:])
                dma.dma_start(out=agin_k[512 + 128 * j:512 + 128 * (j + 1), :],
                              in_=o2[:])

            # V projection (natural layout) -> agin_v
            for sc in range(4):
                for vc in range(2):
                    ps = ps_v.tile([128, LS], fp32, tag="psv")
                    for d8 in range(8):
                        wtile = w_pool.tile([128, LS], bf16, tag="wv")
                        dma.dma_start(
                            out=wtile[:],
                            in_=wqkv[128 * d8:128 * (d8 + 1),
                                     2048 + 512 * vc:2048 + 512 * (vc + 1)],
                        )
                        nc.tensor.matmul(
                            ps[:],
                            xT_sb[:, d8, 128 * sc:128 * (sc + 1)],
                            wtile[:],
                            start=(d8 == 0), stop=(d8 == 7),
                        )
                    vt = vcp.tile([128, LS], bf16, tag="vcp")
                    nc.scalar.activation(vt[:], ps[:], Copy)
                    dma.dma_start(
                        out=agin_v[128 * sc:128 * (sc + 1),
                                   512 * vc:512 * (vc + 1)],
                        in_=vt[:],
                    )

            # ---- all-gather K and V across the batch group ----
            nc.gpsimd.collective_compute(
                "AllGather",
                mybir.AluOpType.bypass,
                replica_groups=[[0, 1, 2, 3], [4, 5, 6, 7]],
                ins=[agin[:].opt()],
                outs=[agout[:].opt()],
            )

            # Q projection + rope -> qattn (overlaps the collective)
            for j in range(4):
                ps1 = proj_T(0 + 128 * j)             # q_x1 chunk j
                ps2 = proj_T(512 + 128 * j)           # q_x2 chunk j
                o1, o2 = rope_pair(ps1, ps2)
                for a in range(4):
                    h = 4 * j + a
                    dma.dma_start(
                        out=qattn[0:32, LS * h:LS * (h + 1)],
                        in_=o1[32 * a:32 * (a + 1), :],
                    )
                    dma.dma_start(
                        out=qattn[32:64, LS * h:LS * (h + 1)],
                        in_=o2[32 * a:32 * (a + 1), :],
                    )

        if stage == 1:
            # debug: consume agout directly
            with tc.tile_pool(name="dbg", bufs=2) as dbg:
                t = dbg.tile([128, LS], bf16, name="dbgt")
                dma.dma_start(out=t[:], in_=agout[0:128, :])
                o = dbg.tile([128, LS], fp32, name="dbgo")
                nc.vector.tensor_copy(o[:], t[:])
                dma.dma_start(out=outT[0:128, :], in_=o[:])
            for f in reversed(_keep):
                f()
            nc.compile()
            return nc

        # ================= phase 2: attention =============================
        with (
            tc.tile_pool(name="ps_sc", bufs=1, space="PSUM") as ps_sc,
            tc.tile_pool(name="ps_out", bufs=4, space="PSUM") as ps_out,
            tc.tile_pool(name="kh_pool", bufs=5) as kh_pool,
            tc.tile_pool(name="vo_pool", bufs=5) as vo_pool,
            tc.tile_pool(name="p_pool", bufs=2) as p_pool,
            tc.tile_pool(name="fin", bufs=4) as fin,
        ):
            for hg in range(1 if stage == 2 else 4):  # head groups of 4
                heads = [4 * hg + i for i in range(4)]
                kh = {}
                vo = {}
                for h in heads:
                    kht = kh_pool.tile([64, S], bf16, tag="kh")
                    for rr in range(4):
                        blk = agout_k_rr(rr)
                        dma.dma_start(
                            out=kht[0:32, 512 * rr:512 * (rr + 1)],
                            in_=blk[32 * h:32 * h + 32, :],
                        )
                        dma.dma_start(
                            out=kht[32:64, 512 * rr:512 * (rr + 1)],
                            in_=blk[512 + 32 * h:512 + 32 * h + 32, :],
                        )
                    kh[h] = kht
                    vot = vo_pool.tile([128, 16, 65], bf16, tag="vo")
                    for kb in range(16):
                        rr, s0 = divmod(kb, 4)
                        dma.dma_start(
                            out=vot[:, kb, 0:64],
                            in_=agout_v_rr(rr)[128 * s0:128 * (s0 + 1),
                                               64 * h:64 * (h + 1)],
                        )
                    nc.vector.memset(vot[:, :, 64], 1.0)
                    vo[h] = vot

                outp = {h: ps_out.tile([128, LS], fp32, tag="pso", name=f"outp{h}") for h in heads}

                for kb in range(16):
                    sc_ps = ps_sc.tile([128, 4 * LS], fp32, tag="sc")
                    for i, h in enumerate(heads):
                        nc.tensor.matmul(
                            sc_ps[:, LS * i:LS * (i + 1)],
                            kh[h][:, 128 * kb:128 * (kb + 1)],
                            qattn[:, LS * h:LS * (h + 1)],
                            start=True, stop=True,
                        )
                    p_sb = p_pool.tile([128, 4 * LS], bf16, tag="p")
                    nc.scalar.activation(p_sb[:], sc_ps[:], Exp, scale=0.125)
                    for i, h in enumerate(heads):
                        nc.tensor.matmul(
                            outp[h][0:65, :],
                            vo[h][:, kb, :],
                            p_sb[:, LS * i:LS * (i + 1)],
                            start=(kb == 0), stop=(kb == 15),
                            skip_group_check=True,
                        )

                # finalize: normalize by the softmax denominator (row 64)
                for h in heads:
                    recip = fin.tile([1, LS], fp32, tag="recip")
                    nc.vector.reciprocal(recip[:], outp[h][64:65, :])
                    bcast = fin.tile([64, LS], fp32, tag="bcast")
                    nc.gpsimd.partition_broadcast(bcast[:], recip[:])
                    nc.vector.tensor_mul(
                        attn_n[64 * (h % 2):64 * (h % 2) + 64, h // 2, :],
                        outp[h][0:64, :],
                        bcast[:],
                    )

        # ================= phase 3: output projection =====================
        with (
            tc.tile_pool(name="wo_pool", bufs=3) as wo_pool,
            tc.tile_pool(name="ps_o", bufs=4, space="PSUM") as ps_o,
            tc.tile_pool(name="ocp", bufs=2) as ocp,
        ):
            for dc in range(8):
                ps = ps_o.tile([128, LS], fp32, tag="pso2")
                for t in range(8):
                    wt = wo_pool.tile([128, 128], bf16, tag="wo")
                    dma.dma_start(
                        out=wt[:],
                        in_=wout[128 * t:128 * (t + 1), 128 * dc:128 * (dc + 1)],
                    )
                    nc.tensor.matmul(
                        ps[:], wt[:], attn_n[:, t, :],
                        start=(t == 0), stop=(t == 7),
                    )
                ot = ocp.tile([128, LS], fp32, tag="ocp")
                nc.scalar.activation(ot[:], ps[:], Copy)
                dma.dma_start(out=outT[128 * dc:128 * (dc + 1), :], in_=ot[:])

        for f in reversed(_keep):
            f()

    nc.compile()
    return nc


def _host_prep(inputs, positions, w_in, w_out):
    inputs = np.asarray(inputs, np.float32)
    positions = np.asarray(positions)
    w_in = np.asarray(w_in, np.float32)
    w_out = np.asarray(w_out, np.float32)

    perm = np.empty(3 * D, dtype=np.int64)
    ar32, ar64 = np.arange(32), np.arange(64)
    for h in range(H):
        perm[32 * h:32 * h + 32] = 192 * h + ar32
        perm[512 + 32 * h:512 + 32 * h + 32] = 192 * h + 32 + ar32
        perm[1024 + 32 * h:1024 + 32 * h + 32] = 192 * h + 64 + ar32
        perm[1536 + 32 * h:1536 + 32 * h + 32] = 192 * h + 96 + ar32
        perm[2048 + 64 * h:2048 + 64 * h + 64] = 192 * h + 128 + ar64
    wqkv = np.ascontiguousarray(w_in[:, perm]).astype(BF16)
    wout_b = np.ascontiguousarray(w_out).astype(BF16)

    inv_freq = (1.0 / (ROPE_THETA ** (np.arange(32, dtype=np.float32) / 32)))

    in_maps = []
    for c in range(NC_):
        b, r = divmod(c, 4)
        sl = slice(LS * r, LS * (r + 1))
        xT = np.ascontiguousarray(inputs[b, sl, :].T).astype(BF16)
        ang = positions[b, sl].astype(np.float32)[None, :] * inv_freq[:, None]
        cosr = np.ascontiguousarray(np.tile(np.cos(ang), (4, 1))).astype(BF16)
        sinr = np.ascontiguousarray(np.tile(np.sin(ang), (4, 1))).astype(BF16)
        in_maps.append({
            "xT": xT, "wqkv": wqkv, "wout": wout_b, "cosr": cosr, "sinr": sinr,
        })
    return in_maps


def kernel(inputs, positions, w_in, w_out, _trace=False):
    global _COMPILED
    from concourse.bass_utils import run_bass_kernel_spmd

    if _COMPILED is None:
        _COMPILED = _build()
    nc = _COMPILED

    in_maps = _host_prep(inputs, positions, w_in, w_out)
    res = run_bass_kernel_spmd(
        nc, in_maps, core_ids=list(range(NC_)), trace=_trace
    )
    kernel.last_results = res

    out = np.zeros((B, S, D), np.float32)
    for c in range(NC_):
        b, r = divmod(c, 4)
        out[b, LS * r:LS * (r + 1), :] = res.results[c]["outT"].T
    return out



# revision 11
# speedup vs baseline: 2.5261x; 2.5261x over previous
"""Distributed Trainium2 kernel for the AttentionBlock problem.

Sharding (v2): tensor-parallel over heads for QKV+attention (each of the 8
cores owns 2 heads for both batches), sequence-parallel for the V projection
and the output projection (each core owns one 512-row block of the flattened
(B*S) dimension).  Two small (1 MB) AllToAll collectives glue the layouts
together:

  1. V is projected seq-parallel (wide, efficient matmuls), then AllToAll'd
     so every core holds V for its own 2 heads over all 4096 rows.  This
     collective overlaps the Q/K projection + RoPE.
  2. After attention, normalized head outputs are AllToAll'd so every core
     holds all 1024 head-dims for its own 512 rows, then applies the full
     output projection locally.  Outputs concatenate on the host.

Device notes:
- All matmul inputs are bf16, PSUM accumulates f32.
- Attention scores are computed transposed (k on partitions, q free) so the
  softmax exp feeds the PV matmul directly; the softmax denominator comes
  from a leading all-ones column prepended to each head's V block.
- Score matmuls for the two batches of a head are row-tiled into the PE
  array concurrently (each uses 64 of the 128 contraction rows).
- The exp runs on the scalar engine at [128,1024] per score block; with
  2-deep score PSUM and 4-deep output PSUM this fits exactly in 8 banks.
"""

import numpy as np
import ml_dtypes

BF16 = ml_dtypes.bfloat16
H, HD, D, B, S = 16, 64, 1024, 2, 2048
LS = 512            # seq rows per core for V / out projection
NC_ = 8
GS = B * S          # 4096 flattened rows
ROPE_THETA = 10000.0

_COMPILED = None


def _build(stage=3):
    import concourse.bass as bass
    import concourse.mybir as mybir
    import concourse.tile as tile
    from concourse import bacc

    fp32 = mybir.dt.float32
    bf16 = mybir.dt.bfloat16

    nc = bacc.Bacc(
        "TRN2", target_bir_lowering=False, debug=False, num_devices=NC_
    )

    xTm = nc.dram_tensor("xTm", [D, LS], bf16, kind="ExternalInput")
    xT = nc.dram_tensor("xT", [D, GS], bf16, kind="ExternalInput")
    wqkv = nc.dram_tensor("wqkv", [D, 256], bf16, kind="ExternalInput")
    wv = nc.dram_tensor("wv", [D, D], bf16, kind="ExternalInput")
    wout = nc.dram_tensor("wout", [D, D], bf16, kind="ExternalInput")
    cosr = nc.dram_tensor("cosr", [128, GS], bf16, kind="ExternalInput")
    sinr = nc.dram_tensor("sinr", [128, GS], bf16, kind="ExternalInput")
    outT = nc.dram_tensor("outT", [D, LS], fp32, kind="ExternalOutput")

    Exp = mybir.ActivationFunctionType.Exp
    Copy = mybir.ActivationFunctionType.Copy
    GROUPS = [[0, 1, 2, 3, 4, 5, 6, 7]]

    with tile.TileContext(nc) as tc:
        dma = nc.default_dma_engine
        _keep = []

        def _single(*args, **kwargs):
            t, f = tc.tile(*args, **kwargs)
            _keep.append(f)
            return t

        # ---- persistent SBUF tensors ----
        xTm_sb = _single([128, 8, LS], bf16, name="xTm_sb")
        wv_sb = _single([128, 8, D], bf16, name="wv_sb")
        wqkv_sb = _single([128, 8, 256], bf16, name="wqkv_sb")
        cos_sb = _single([128, GS], bf16, name="cos_sb")
        sin_sb = _single([128, GS], bf16, name="sin_sb")
        xT_sb = _single([128, 8, GS], bf16, name="xT_sb")
        wout_sb = _single([128, 8, D], bf16, name="wout_sb")
        o1_all = _single([128, GS], bf16, name="o1_all")
        o2_all = _single([128, GS], bf16, name="o2_all")
        qp = [_single([128, S], bf16, name=f"qp{h}") for h in range(2)]
        khp = [_single([128, S], bf16, name=f"khp{h}") for h in range(2)]
        vo_all = _single([128, 32, 130], bf16, name="vo_all")
        attn_T = _single([128, 8, LS], bf16, name="attn_T")

        # ---- internal DRAM for the collectives ----
        # a2aV: rank-block r = rows [512r:512(r+1)] = (my 512 seq rows) x
        # (128 v-cols of heads 2r, 2r+1).  a2aA: rank-block r = rows
        # [128r:128(r+1)] = (my 2 heads' 128 dims) x (512 q rows of block r).
        a2aV_in = _single([NC_ * LS, 128], bf16, space="DRAM", name="a2aV_in")
        a2aV_out = _single([NC_ * LS, 128], bf16, space="DRAM",
                           name="a2aV_out")
        a2aA_in = _single([NC_ * 128, LS], bf16, space="DRAM", name="a2aA_in")
        a2aA_out = _single([NC_ * 128, LS], bf16, space="DRAM",
                           name="a2aA_out")

        # ---- input DMAs, in pipeline order ----
        for half in range(2):
            dma.dma_start(
                out=wv_sb[:, :, 512 * half:512 * (half + 1)],
                in_=wv[:, 512 * half:512 * (half + 1)].rearrange(
                    "(c p) v -> p c v", p=128),
            )
        dma.dma_start(out=xTm_sb[:],
                      in_=xTm[:].rearrange("(c p) s -> p c s", p=128))
        dma.dma_start(out=wqkv_sb[:],
                      in_=wqkv[:].rearrange("(c p) k -> p c k", p=128))
        dma.dma_start(out=cos_sb[:, 0:S], in_=cosr[:, 0:S])
        dma.dma_start(out=sin_sb[:, 0:S], in_=sinr[:, 0:S])
        for q4 in range(2):
            dma.dma_start(
                out=xT_sb[:, :, S * q4:S * (q4 + 1)],
                in_=xT[:, S * q4:S * (q4 + 1)].rearrange(
                    "(c p) s -> p c s", p=128),
            )
        dma.dma_start(out=cos_sb[:, S:GS], in_=cosr[:, S:GS])
        dma.dma_start(out=sin_sb[:, S:GS], in_=sinr[:, S:GS])
        for half in range(2):
            dma.dma_start(
                out=wout_sb[:, :, 512 * half:512 * (half + 1)],
                in_=wout[:, 512 * half:512 * (half + 1)].rearrange(
                    "(c p) v -> p c v", p=128),
            )

        # ================= phase 1: V projection (seq-parallel) ============
        with (
            tc.tile_pool(name="ps_v", bufs=4, space="PSUM") as ps_v,
            tc.tile_pool(name="vcp", bufs=4) as vcp,
        ):
            for sc in range(4):
                for vc in range(2):
                    ps = ps_v.tile([128, 512], fp32, tag="psv")
                    for d8 in range(8):
                        nc.tensor.matmul(
                            ps[:],
                            xTm_sb[:, d8, 128 * sc:128 * (sc + 1)],
                            wv_sb[:, d8, 512 * vc:512 * (vc + 1)],
                            start=(d8 == 0), stop=(d8 == 7),
                        )
                    vt = vcp.tile([128, 512], bf16, tag="vcp")
                    nc.scalar.activation(vt[:], ps[:], Copy)
                    for r4 in range(4):
                        row0 = LS * (4 * vc + r4) + 128 * sc
                        dma.dma_start(
                            out=a2aV_in[row0:row0 + 128, :],
                            in_=vt[:, 128 * r4:128 * (r4 + 1)],
                        )

        nc.gpsimd.collective_compute(
            "AllToAll",
            mybir.AluOpType.bypass,
            replica_groups=GROUPS,
            ins=[a2aV_in[:].opt()],
            outs=[a2aV_out[:].opt()],
        )

        # vo_all[p, kbg, [v_h0(64) | 1 | v_h1(64) | 1]]
        nc.vector.memset(vo_all[:, :, 64:65], 1.0)
        nc.vector.memset(vo_all[:, :, 129:130], 1.0)
        for r in range(NC_):
            src = a2aV_out[LS * r:LS * (r + 1), :].rearrange(
                "(k p) v -> p k v", p=128)
            dma.dma_start(out=vo_all[:, 4 * r:4 * (r + 1), 0:64],
                          in_=src[:, :, 0:64])
            dma.dma_start(out=vo_all[:, 4 * r:4 * (r + 1), 65:129],
                          in_=src[:, :, 64:128])

        # ================= phase 2: Q/K projection + rope ==================
        with (
            tc.tile_pool(name="ps_kq", bufs=4, space="PSUM") as ps_kq,
            tc.tile_pool(name="rope_t", bufs=4) as rope_t,
        ):
            for s8 in range(8):
                sl = slice(512 * s8, 512 * (s8 + 1))
                ps1 = ps_kq.tile([128, 512], fp32, tag="pskq")
                for d8 in range(8):
                    nc.tensor.matmul(
                        ps1[:], wqkv_sb[:, d8, 0:128], xT_sb[:, d8, sl],
                        start=(d8 == 0), stop=(d8 == 7),
                    )
                ps2 = ps_kq.tile([128, 512], fp32, tag="pskq")
                for d8 in range(8):
                    nc.tensor.matmul(
                        ps2[:], wqkv_sb[:, d8, 128:256], xT_sb[:, d8, sl],
                        start=(d8 == 0), stop=(d8 == 7),
                    )
                cs, sn = cos_sb[:, sl], sin_sb[:, sl]
                t1 = rope_t.tile([128, 512], bf16, tag="rt")
                t2 = rope_t.tile([128, 512], bf16, tag="rt")
                nc.vector.tensor_mul(t1[:], ps1[:], cs)
                nc.vector.tensor_mul(t2[:], ps2[:], sn)
                nc.vector.tensor_sub(o1_all[:, sl], t1[:], t2[:])
                nc.vector.tensor_mul(t1[:], ps1[:], sn)
                nc.vector.tensor_mul(t2[:], ps2[:], cs)
                nc.vector.tensor_add(o2_all[:, sl], t1[:], t2[:])

        # assemble per-head q^T / k^T pair tiles (b0 rows 0:64, b1 rows 64:128)
        for hl in range(2):
            for bb in range(2):
                bsl = slice(S * bb, S * (bb + 1))
                dma.dma_start(out=qp[hl][64 * bb:64 * bb + 32, :],
                              in_=o1_all[32 * hl:32 * (hl + 1), bsl])
                dma.dma_start(out=qp[hl][64 * bb + 32:64 * bb + 64, :],
                              in_=o2_all[32 * hl:32 * (hl + 1), bsl])
                dma.dma_start(out=khp[hl][64 * bb:64 * bb + 32, :],
                              in_=o1_all[64 + 32 * hl:64 + 32 * (hl + 1), bsl])
                dma.dma_start(out=khp[hl][64 * bb + 32:64 * bb + 64, :],
                              in_=o2_all[64 + 32 * hl:64 + 32 * (hl + 1), bsl])

        if stage == 1:
            with tc.tile_pool(name="dbg", bufs=2) as dbg:
                o = dbg.tile([128, LS], fp32, name="dbgo")
                nc.vector.tensor_copy(o[:, 0:64], vo_all[:, 0, 0:64])
                nc.vector.tensor_copy(o[:, 64:128], vo_all[:, 17, 65:129])
                nc.vector.memset(o[:, 128:LS], 0.0)
                dma.dma_start(out=outT[0:128, :], in_=o[:])
                o2 = dbg.tile([128, LS], fp32, name="dbgo2")
                nc.vector.tensor_copy(o2[:], qp[0][:, 0:LS])
                dma.dma_start(out=outT[128:256, :], in_=o2[:])
                o3 = dbg.tile([128, LS], fp32, name="dbgo3")
                nc.vector.tensor_copy(o3[:], khp[0][:, 0:LS])
                dma.dma_start(out=outT[256:384, :], in_=o3[:])
            for f in reversed(_keep):
                f()
            nc.compile()
            return nc

        # ================= phase 3: attention ==============================
        with (
            tc.tile_pool(name="ps_sc", bufs=2, space="PSUM") as ps_sc,
            tc.tile_pool(name="ps_out", bufs=4, space="PSUM") as ps_out,
            tc.tile_pool(name="p_pool", bufs=3) as p_pool,
            tc.tile_pool(name="fin", bufs=4) as fin,
        ):
            for hl in range(2):
                for j in range(4):
                    qsl = slice(512 * j, 512 * (j + 1))
                    outp = [ps_out.tile([128, 512], fp32, tag="pso",
                                        name=f"outp{hl}_{j}_{u}")
                            for u in range(2)]
                    for kb in range(16):
                        sc_ps = ps_sc.tile([128, 1024], fp32, tag="sc")
                        ksl = slice(128 * kb, 128 * (kb + 1))
                        for u in range(2):
                            nc.tensor.matmul(
                                sc_ps[:, 512 * u:512 * (u + 1)],
                                khp[hl][64 * u:64 * (u + 1), ksl],
                                qp[hl][64 * u:64 * (u + 1), qsl],
                                start=True, stop=True,
                            )
                        p_sb = p_pool.tile([128, 1024], bf16, tag="p")
                        nc.scalar.activation(p_sb[:], sc_ps[:], Exp,
                                             scale=0.125)
                        for u in range(2):
                            nc.tensor.matmul(
                                outp[u][0:65, :],
                                vo_all[:, 16 * u + kb, 65 * hl:65 * (hl + 1)],
                                p_sb[:, 512 * u:512 * (u + 1)],
                                start=(kb == 0), stop=(kb == 15),
                                skip_group_check=True,
                            )
                    for u in range(2):
                        recip = fin.tile([1, 512], fp32, tag="recip")
                        nc.vector.reciprocal(recip[:], outp[u][64:65, :])
                        bcast = fin.tile([64, 512], fp32, tag="bcast")
                        nc.gpsimd.partition_broadcast(bcast[:], recip[:])
                        att = fin.tile([64, 512], bf16, tag="att")
                        nc.vector.tensor_mul(att[:], outp[u][0:64, :],
                                             bcast[:])
                        row0 = 128 * (4 * u + j) + 64 * hl
                        dma.dma_start(
                            out=a2aA_in[row0:row0 + 64, :],
                            in_=att[:],
                        )

        nc.gpsimd.collective_compute(
            "AllToAll",
            mybir.AluOpType.bypass,
            replica_groups=GROUPS,
            ins=[a2aA_in[:].opt()],
            outs=[a2aA_out[:].opt()],
        )
        for t in range(8):
            dma.dma_start(out=attn_T[:, t, :],
                          in_=a2aA_out[128 * t:128 * (t + 1), :])

        # ================= phase 4: output projection ======================
        with (
            tc.tile_pool(name="ps_o", bufs=4, space="PSUM") as ps_o,
            tc.tile_pool(name="ocp", bufs=4) as ocp,
        ):
            for oc in range(8):
                ps = ps_o.tile([128, 512], fp32, tag="pso2")
                for t in range(8):
                    nc.tensor.matmul(
                        ps[:], wout_sb[:, t, 128 * oc:128 * (oc + 1)],
                        attn_T[:, t, :],
                        start=(t == 0), stop=(t == 7),
                    )
                ot = ocp.tile([128, 512], fp32, tag="ocp")
                nc.scalar.activation(ot[:], ps[:], Copy)
                dma.dma_start(out=outT[128 * oc:128 * (oc + 1), :], in_=ot[:])

        for f in reversed(_keep):
            f()

    nc.compile()
    return nc


def _host_prep(inputs, positions, w_in, w_out):
    inputs = np.asarray(inputs, np.float32)
    positions = np.asarray(positions)
    w_in = np.asarray(w_in, np.float32)
    w_out = np.asarray(w_out, np.float32)

    x_all = np.concatenate([inputs[0], inputs[1]], axis=0)          # (4096, D)
    xT_full = np.ascontiguousarray(x_all.T).astype(BF16)            # (D, 4096)

    ar32, ar64 = np.arange(32), np.arange(64)
    vcols = np.concatenate([192 * h + 128 + ar64 for h in range(H)])
    wv = np.ascontiguousarray(w_in[:, vcols]).astype(BF16)          # (D, D)
    wout_b = np.ascontiguousarray(w_out).astype(BF16)

    inv_freq = 1.0 / (ROPE_THETA ** (np.arange(32, dtype=np.float32) / 32))
    pos_all = np.concatenate([positions[0], positions[1]]).astype(np.float32)
    ang = pos_all[None, :] * inv_freq[:, None]                      # (32, 4096)
    cosr = np.ascontiguousarray(np.tile(np.cos(ang), (4, 1))).astype(BF16)
    sinr = np.ascontiguousarray(np.tile(np.sin(ang), (4, 1))).astype(BF16)

    in_maps = []
    for c in range(NC_):
        b, j = divmod(c, 4)
        xTm = np.ascontiguousarray(
            inputs[b, LS * j:LS * (j + 1), :].T).astype(BF16)
        H0, H1 = 2 * c, 2 * c + 1
        cols = np.concatenate([
            192 * H0 + ar32, 192 * H1 + ar32,            # q_x1 h0, h1
            192 * H0 + 64 + ar32, 192 * H1 + 64 + ar32,  # k_x1 h0, h1
            192 * H0 + 32 + ar32, 192 * H1 + 32 + ar32,  # q_x2 h0, h1
            192 * H0 + 96 + ar32, 192 * H1 + 96 + ar32,  # k_x2 h0, h1
        ])
        wqkv = np.ascontiguousarray(w_in[:, cols]).astype(BF16)
        in_maps.append({
            "xTm": xTm, "xT": xT_full, "wqkv": wqkv, "wv": wv,
            "wout": wout_b, "cosr": cosr, "sinr": sinr,
        })
    return in_maps


def kernel(inputs, positions, w_in, w_out, _trace=False):
    global _COMPILED
    from concourse.bass_utils import run_bass_kernel_spmd

    if _COMPILED is None:
        _COMPILED = _build()
    nc = _COMPILED

    in_maps = _host_prep(inputs, positions, w_in, w_out)
    res = run_bass_kernel_spmd(
        nc, in_maps, core_ids=list(range(NC_)), trace=_trace
    )
    kernel.last_results = res

    out = np.zeros((B, S, D), np.float32)
    for c in range(NC_):
        b, j = divmod(c, 4)
        out[b, LS * j:LS * (j + 1), :] = res.results[c]["outT"].T
    return out


# revision 24
# speedup vs baseline: 2.6009x; 1.0296x over previous
"""Distributed Trainium2 kernel for the AttentionBlock problem.

Sharding (v2): tensor-parallel over heads for QKV+attention (each of the 8
cores owns 2 heads for both batches), sequence-parallel for the V projection
and the output projection (each core owns one 512-row block of the flattened
(B*S) dimension).  Two small (1 MB) AllToAll collectives glue the layouts
together:

  1. V is projected seq-parallel (wide, efficient matmuls), then AllToAll'd
     so every core holds V for its own 2 heads over all 4096 rows.  This
     collective overlaps the Q/K projection + RoPE.
  2. After attention, normalized head outputs are AllToAll'd so every core
     holds all 1024 head-dims for its own 512 rows, then applies the full
     output projection locally.  Outputs concatenate on the host.

Device notes:
- All matmul inputs are bf16, PSUM accumulates f32.
- Attention scores are computed transposed (k on partitions, q free) so the
  softmax exp feeds the PV matmul directly; the softmax denominator comes
  from a leading all-ones column prepended to each head's V block.
- Score matmuls for the two batches of a head are row-tiled into the PE
  array concurrently (each uses 64 of the 128 contraction rows).
- The exp runs on the scalar engine at [128,1024] per score block; with
  2-deep score PSUM and 4-deep output PSUM this fits exactly in 8 banks.
"""

import numpy as np
import ml_dtypes

BF16 = ml_dtypes.bfloat16
H, HD, D, B, S = 16, 64, 1024, 2, 2048
LS = 512            # seq rows per core for V / out projection
NC_ = 8
GS = B * S          # 4096 flattened rows
ROPE_THETA = 10000.0

_COMPILED = None


def _build(stage=3):
    import concourse.bass as bass
    import concourse.mybir as mybir
    import concourse.tile as tile
    from concourse import bacc

    fp32 = mybir.dt.float32
    bf16 = mybir.dt.bfloat16

    nc = bacc.Bacc(
        "TRN2", target_bir_lowering=False, debug=False, num_devices=NC_
    )

    xTm = nc.dram_tensor("xTm", [D, LS], bf16, kind="ExternalInput")
    xT = nc.dram_tensor("xT", [D, GS], bf16, kind="ExternalInput")
    wqkv = nc.dram_tensor("wqkv", [D, 256], bf16, kind="ExternalInput")
    wv = nc.dram_tensor("wv", [D, D], bf16, kind="ExternalInput")
    wout = nc.dram_tensor("wout", [D, D], bf16, kind="ExternalInput")
    cosr = nc.dram_tensor("cosr", [128, GS], bf16, kind="ExternalInput")
    sinr = nc.dram_tensor("sinr", [128, GS], bf16, kind="ExternalInput")
    outT = nc.dram_tensor("outT", [D, LS], fp32, kind="ExternalOutput")

    Exp = mybir.ActivationFunctionType.Exp
    Copy = mybir.ActivationFunctionType.Copy
    GROUPS = [[0, 1, 2, 3, 4, 5, 6, 7]]

    with tile.TileContext(nc) as tc:
        dma = nc.default_dma_engine
        _keep = []

        def _single(*args, **kwargs):
            t, f = tc.tile(*args, **kwargs)
            _keep.append(f)
            return t

        # ---- persistent SBUF tensors ----
        xTm_sb = _single([128, 8, LS], bf16, name="xTm_sb")
        wv_sb = _single([128, 8, D], bf16, name="wv_sb")
        wqkv_sb = _single([128, 8, 256], bf16, name="wqkv_sb")
        cos_sb = _single([128, GS], bf16, name="cos_sb")
        sin_sb = _single([128, GS], bf16, name="sin_sb")
        wout_sb = _single([128, 8, D], bf16, name="wout_sb")
        o1_all = _single([128, GS], bf16, name="o1_all")
        o2_all = _single([128, GS], bf16, name="o2_all")
        # per-batch head-pair tiles: rows 0:64 = head h0, 64:128 = head h1
        qp = [_single([128, S], bf16, name=f"qp{b}") for b in range(2)]
        khp = [_single([128, S], bf16, name=f"khp{b}") for b in range(2)]
        vo_all = _single([128, 32, 130], bf16, name="vo_all")
        attn_T = _single([128, 8, LS], bf16, name="attn_T")

        # ---- internal DRAM for the collectives ----
        # a2aV: rank-block r = rows [512r:512(r+1)] = (my 512 seq rows) x
        # (128 v-cols of heads 2r, 2r+1).  a2aA: rank-block r = rows
        # [128r:128(r+1)] = (my 2 heads' 128 dims) x (512 q rows of block r).
        a2aV_in = _single([NC_ * LS, 128], bf16, space="DRAM", name="a2aV_in")
        a2aV_out = _single([NC_ * LS, 128], bf16, space="DRAM",
                           name="a2aV_out")
        a2aA_in = _single([NC_ * 128, LS], bf16, space="DRAM", name="a2aA_in")
        a2aA_out = _single([NC_ * 128, LS], bf16, space="DRAM",
                           name="a2aA_out")

        # ---- input DMAs, in pipeline order ----
        for half in range(2):
            dma.dma_start(
                out=wv_sb[:, :, 512 * half:512 * (half + 1)],
                in_=wv[:, 512 * half:512 * (half + 1)].rearrange(
                    "(c p) v -> p c v", p=128),
            )
        dma.dma_start(out=xTm_sb[:],
                      in_=xTm[:].rearrange("(c p) s -> p c s", p=128))
        dma.dma_start(out=wqkv_sb[:],
                      in_=wqkv[:].rearrange("(c p) k -> p c k", p=128))
        dma.dma_start(out=cos_sb[:, 0:S], in_=cosr[:, 0:S])
        dma.dma_start(out=sin_sb[:, 0:S], in_=sinr[:, 0:S])
        dma.dma_start(out=cos_sb[:, S:GS], in_=cosr[:, S:GS])
        dma.dma_start(out=sin_sb[:, S:GS], in_=sinr[:, S:GS])

        xq_pool_cm = tc.tile_pool(name="xq_pool", bufs=2)
        xq_pool = xq_pool_cm.__enter__()
        # Pre-issue the first two xT quarters on the scalar-engine HWDGE
        # queue so the Q/K projection input never queues behind the V-phase
        # writes on the sync queue.
        xq_tiles = []
        for q4 in range(2):
            xq = xq_pool.tile([128, 8, 1024], bf16, tag="xq")
            nc.scalar.dma_start(
                out=xq[:],
                in_=xT[:, 1024 * q4:1024 * (q4 + 1)].rearrange(
                    "(c p) s -> p c s", p=128),
            )
            xq_tiles.append(xq)

        # ================= phase 1: V projection (seq-parallel) ============
        with (
            tc.tile_pool(name="ps_v", bufs=4, space="PSUM") as ps_v,
            tc.tile_pool(name="vcp", bufs=4) as vcp,
        ):
            for sc in range(4):
                for vc in range(2):
                    ps = ps_v.tile([128, 512], fp32, tag="psv")
                    for d8 in range(8):
                        nc.tensor.matmul(
                            ps[:],
                            xTm_sb[:, d8, 128 * sc:128 * (sc + 1)],
                            wv_sb[:, d8, 512 * vc:512 * (vc + 1)],
                            start=(d8 == 0), stop=(d8 == 7),
                        )
                    vt = vcp.tile([128, 512], bf16, tag="vcp")
                    nc.scalar.activation(vt[:], ps[:], Copy)
                    for r4 in range(4):
                        row0 = LS * (4 * vc + r4) + 128 * sc
                        dma.dma_start(
                            out=a2aV_in[row0:row0 + 128, :],
                            in_=vt[:, 128 * r4:128 * (r4 + 1)],
                        )

        nc.gpsimd.collective_compute(
            "AllToAll",
            mybir.AluOpType.bypass,
            replica_groups=GROUPS,
            ins=[a2aV_in[:].opt()],
            outs=[a2aV_out[:].opt()],
        )

        # vo_all[p, kbg, [v_h0(64) | 1 | v_h1(64) | 1]]
        nc.vector.memset(vo_all[:, :, 64:65], 1.0)
        nc.vector.memset(vo_all[:, :, 129:130], 1.0)

        # ================= phase 2: Q/K projection + rope ==================
        with (
            tc.tile_pool(name="ps_kq", bufs=4, space="PSUM") as ps_kq,
            tc.tile_pool(name="rope_t", bufs=4) as rope_t,
        ):
            for q4 in range(4):
                if q4 < 2:
                    xq = xq_tiles[q4]
                else:
                    xq = xq_pool.tile([128, 8, 1024], bf16, tag="xq")
                    dma.dma_start(
                        out=xq[:],
                        in_=xT[:, 1024 * q4:1024 * (q4 + 1)].rearrange(
                            "(c p) s -> p c s", p=128),
                    )
                for s2 in range(2):
                    sl = slice(1024 * q4 + 512 * s2, 1024 * q4 + 512 * (s2 + 1))
                    xsl = slice(512 * s2, 512 * (s2 + 1))
                    ps1 = ps_kq.tile([128, 512], fp32, tag="pskq")
                    for d8 in range(8):
                        nc.tensor.matmul(
                            ps1[:], wqkv_sb[:, d8, 0:128], xq[:, d8, xsl],
                            start=(d8 == 0), stop=(d8 == 7),
                        )
                    ps2 = ps_kq.tile([128, 512], fp32, tag="pskq")
                    for d8 in range(8):
                        nc.tensor.matmul(
                            ps2[:], wqkv_sb[:, d8, 128:256], xq[:, d8, xsl],
                            start=(d8 == 0), stop=(d8 == 7),
                        )
                    cs, sn = cos_sb[:, sl], sin_sb[:, sl]
                    t1 = rope_t.tile([128, 512], bf16, tag="rt")
                    t2 = rope_t.tile([128, 512], bf16, tag="rt")
                    nc.vector.tensor_mul(t1[:], ps1[:], cs)
                    nc.vector.tensor_mul(t2[:], ps2[:], sn)
                    nc.vector.tensor_sub(o1_all[:, sl], t1[:], t2[:])
                    nc.vector.tensor_mul(t1[:], ps1[:], sn)
                    nc.vector.tensor_mul(t2[:], ps2[:], cs)
                    nc.vector.tensor_add(o2_all[:, sl], t1[:], t2[:])
                if q4 % 2 == 1:
                    # batch bb fully roped: assemble its head-pair tiles
                    # qp/khp rows: 0:32 h0_x1', 32:64 h0_x2', 64:96 h1_x1',
                    # 96:128 h1_x2'
                    bb = q4 // 2
                    bsl = slice(S * bb, S * (bb + 1))
                    for hl in range(2):
                        dma.dma_start(
                            out=qp[bb][64 * hl:64 * hl + 32, :],
                            in_=o1_all[32 * hl:32 * (hl + 1), bsl])
                        dma.dma_start(
                            out=qp[bb][64 * hl + 32:64 * hl + 64, :],
                            in_=o2_all[32 * hl:32 * (hl + 1), bsl])
                        dma.dma_start(
                            out=khp[bb][64 * hl:64 * hl + 32, :],
                            in_=o1_all[64 + 32 * hl:64 + 32 * (hl + 1), bsl])
                        dma.dma_start(
                            out=khp[bb][64 * hl + 32:64 * hl + 64, :],
                            in_=o2_all[64 + 32 * hl:64 + 32 * (hl + 1), bsl])

        xq_pool_cm.__exit__(None, None, None)

        # vo loads issued after the assembly DMAs so their wait on the V
        # collective cannot block the (earlier-needed) assembly transfers
        # in the queue.
        for r in range(NC_):
            src = a2aV_out[LS * r:LS * (r + 1), :].rearrange(
                "(k p) v -> p k v", p=128)
            dma.dma_start(out=vo_all[:, 4 * r:4 * (r + 1), 0:64],
                          in_=src[:, :, 0:64])
            dma.dma_start(out=vo_all[:, 4 * r:4 * (r + 1), 65:129],
                          in_=src[:, :, 64:128])

        # wout load issued here: the queue reaches it right after assembly
        # (~mid-kernel), well before the output projection needs it.
        for half in range(2):
            dma.dma_start(
                out=wout_sb[:, :, 512 * half:512 * (half + 1)],
                in_=wout[:, 512 * half:512 * (half + 1)].rearrange(
                    "(c p) v -> p c v", p=128),
            )

        if stage == 1:
            with tc.tile_pool(name="dbg", bufs=2) as dbg:
                o = dbg.tile([128, LS], fp32, name="dbgo")
                nc.vector.tensor_copy(o[:, 0:64], vo_all[:, 0, 0:64])
                nc.vector.tensor_copy(o[:, 64:128], vo_all[:, 17, 65:129])
                nc.vector.memset(o[:, 128:LS], 0.0)
                dma.dma_start(out=outT[0:128, :], in_=o[:])
                o2 = dbg.tile([128, LS], fp32, name="dbgo2")
                nc.vector.tensor_copy(o2[:], qp[0][:, 0:LS])
                dma.dma_start(out=outT[128:256, :], in_=o2[:])
                o3 = dbg.tile([128, LS], fp32, name="dbgo3")
                nc.vector.tensor_copy(o3[:], khp[0][:, 0:LS])
                dma.dma_start(out=outT[256:384, :], in_=o3[:])
            for f in reversed(_keep):
                f()
            nc.compile()
            return nc

        # ================= phase 3: attention ==============================
        with (
            tc.tile_pool(name="ps_sc", bufs=2, space="PSUM") as ps_sc,
            tc.tile_pool(name="ps_out", bufs=4, space="PSUM") as ps_out,
            tc.tile_pool(name="p_pool", bufs=24) as p_pool,
            tc.tile_pool(name="fin", bufs=4) as fin,
        ):
            for bb in range(2):
                for j in range(4):
                    qsl = slice(512 * j, 512 * (j + 1))
                    outp = [ps_out.tile([128, 512], fp32, tag="pso",
                                        name=f"outp{bb}_{j}_{u}")
                            for u in range(2)]
                    for kb in range(16):
                        sc_ps = ps_sc.tile([128, 1024], fp32, tag="sc")
                        ksl = slice(128 * kb, 128 * (kb + 1))
                        for u in range(2):
                            nc.tensor.matmul(
                                sc_ps[:, 512 * u:512 * (u + 1)],
                                khp[bb][64 * u:64 * (u + 1), ksl],
                                qp[bb][64 * u:64 * (u + 1), qsl],
                                start=True, stop=True,
                            )
                        p_sb = p_pool.tile([128, 1024], bf16, tag="p")
                        nc.scalar.activation(p_sb[:], sc_ps[:], Exp,
                                             scale=0.125)
                        for u in range(2):
                            nc.tensor.matmul(
                                outp[u][0:65, :],
                                vo_all[:, 16 * bb + kb, 65 * u:65 * (u + 1)],
                                p_sb[:, 512 * u:512 * (u + 1)],
                                start=(kb == 0), stop=(kb == 15),
                                skip_group_check=True,
                            )
                    for u in range(2):
                        recip = fin.tile([1, 512], fp32, tag="recip")
                        nc.vector.reciprocal(recip[:], outp[u][64:65, :])
                        bcast = fin.tile([64, 512], fp32, tag="bcast")
                        nc.gpsimd.partition_broadcast(bcast[:], recip[:])
                        att = fin.tile([64, 512], bf16, tag="att")
                        nc.vector.tensor_mul(att[:], outp[u][0:64, :],
                                             bcast[:])
                        row0 = 128 * (4 * bb + j) + 64 * u
                        dma.dma_start(
                            out=a2aA_in[row0:row0 + 64, :],
                            in_=att[:],
                        )

        nc.gpsimd.collective_compute(
            "AllToAll",
            mybir.AluOpType.bypass,
            replica_groups=GROUPS,
            ins=[a2aA_in[:].opt()],
            outs=[a2aA_out[:].opt()],
        )
        for t in range(8):
            dma.dma_start(out=attn_T[:, t, :],
                          in_=a2aA_out[128 * t:128 * (t + 1), :])

        # ================= phase 4: output projection ======================
        with (
            tc.tile_pool(name="ps_o", bufs=4, space="PSUM") as ps_o,
            tc.tile_pool(name="ocp", bufs=4) as ocp,
        ):
            for oc in range(8):
                ps = ps_o.tile([128, 512], fp32, tag="pso2")
                for t in range(8):
                    nc.tensor.matmul(
                        ps[:], wout_sb[:, t, 128 * oc:128 * (oc + 1)],
                        attn_T[:, t, :],
                        start=(t == 0), stop=(t == 7),
                    )
                ot = ocp.tile([128, 512], fp32, tag="ocp")
                nc.scalar.activation(ot[:], ps[:], Copy)
                dma.dma_start(out=outT[128 * oc:128 * (oc + 1), :], in_=ot[:])

        for f in reversed(_keep):
            f()

    nc.compile()
    return nc


def _host_prep(inputs, positions, w_in, w_out):
    inputs = np.asarray(inputs, np.float32)
    positions = np.asarray(positions)
    w_in = np.asarray(w_in, np.float32)
    w_out = np.asarray(w_out, np.float32)

    x_all = np.concatenate([inputs[0], inputs[1]], axis=0)          # (4096, D)
    xT_full = np.ascontiguousarray(x_all.T).astype(BF16)            # (D, 4096)

    ar32, ar64 = np.arange(32), np.arange(64)
    vcols = np.concatenate([192 * h + 128 + ar64 for h in range(H)])
    wv = np.ascontiguousarray(w_in[:, vcols]).astype(BF16)          # (D, D)
    wout_b = np.ascontiguousarray(w_out).astype(BF16)

    inv_freq = 1.0 / (ROPE_THETA ** (np.arange(32, dtype=np.float32) / 32))
    pos_all = np.concatenate([positions[0], positions[1]]).astype(np.float32)
    ang = pos_all[None, :] * inv_freq[:, None]                      # (32, 4096)
    cosr = np.ascontiguousarray(np.tile(np.cos(ang), (4, 1))).astype(BF16)
    sinr = np.ascontiguousarray(np.tile(np.sin(ang), (4, 1))).astype(BF16)

    in_maps = []
    for c in range(NC_):
        b, j = divmod(c, 4)
        xTm = np.ascontiguousarray(
            inputs[b, LS * j:LS * (j + 1), :].T).astype(BF16)
        H0, H1 = 2 * c, 2 * c + 1
        cols = np.concatenate([
            192 * H0 + ar32, 192 * H1 + ar32,            # q_x1 h0, h1
            192 * H0 + 64 + ar32, 192 * H1 + 64 + ar32,  # k_x1 h0, h1
            192 * H0 + 32 + ar32, 192 * H1 + 32 + ar32,  # q_x2 h0, h1
            192 * H0 + 96 + ar32, 192 * H1 + 96 + ar32,  # k_x2 h0, h1
        ])
        wqkv = np.ascontiguousarray(w_in[:, cols]).astype(BF16)
        in_maps.append({
            "xTm": xTm, "xT": xT_full, "wqkv": wqkv, "wv": wv,
            "wout": wout_b, "cosr": cosr, "sinr": sinr,
        })
    return in_maps


def kernel(inputs, positions, w_in, w_out, _trace=False):
    global _COMPILED
    from concourse.bass_utils import run_bass_kernel_spmd

    if _COMPILED is None:
        _COMPILED = _build()
    nc = _COMPILED

    in_maps = _host_prep(inputs, positions, w_in, w_out)
    res = run_bass_kernel_spmd(
        nc, in_maps, core_ids=list(range(NC_)), trace=_trace
    )
    kernel.last_results = res

    out = np.zeros((B, S, D), np.float32)
    for c in range(NC_):
        b, j = divmod(c, 4)
        out[b, LS * j:LS * (j + 1), :] = res.results[c]["outT"].T
    return out


# revision 31
# speedup vs baseline: 2.6167x; 1.0061x over previous
"""Distributed Trainium2 kernel for the AttentionBlock problem.

Sharding (v2): tensor-parallel over heads for QKV+attention (each of the 8
cores owns 2 heads for both batches), sequence-parallel for the V projection
and the output projection (each core owns one 512-row block of the flattened
(B*S) dimension).  Two small (1 MB) AllToAll collectives glue the layouts
together:

  1. V is projected seq-parallel (wide, efficient matmuls), then AllToAll'd
     so every core holds V for its own 2 heads over all 4096 rows.  This
     collective overlaps the Q/K projection + RoPE.
  2. After attention, normalized head outputs are AllToAll'd so every core
     holds all 1024 head-dims for its own 512 rows, then applies the full
     output projection locally.  Outputs concatenate on the host.

Device notes:
- All matmul inputs are bf16, PSUM accumulates f32.
- Attention scores are computed transposed (k on partitions, q free) so the
  softmax exp feeds the PV matmul directly; the softmax denominator comes
  from a leading all-ones column prepended to each head's V block.
- Score matmuls for the two batches of a head are row-tiled into the PE
  array concurrently (each uses 64 of the 128 contraction rows).
- The exp runs on the scalar engine at [128,1024] per score block; with
  2-deep score PSUM and 4-deep output PSUM this fits exactly in 8 banks.
"""

import numpy as np
import ml_dtypes

BF16 = ml_dtypes.bfloat16
H, HD, D, B, S = 16, 64, 1024, 2, 2048
LS = 512            # seq rows per core for V / out projection
NC_ = 8
GS = B * S          # 4096 flattened rows
ROPE_THETA = 10000.0

_COMPILED = None


def _build(stage=3):
    import concourse.bass as bass
    import concourse.mybir as mybir
    import concourse.tile as tile
    from concourse import bacc

    fp32 = mybir.dt.float32
    bf16 = mybir.dt.bfloat16

    nc = bacc.Bacc(
        "TRN2", target_bir_lowering=False, debug=False, num_devices=NC_
    )

    xT = nc.dram_tensor("xT", [D, GS], bf16, kind="ExternalInput")
    wqkv = nc.dram_tensor("wqkv", [D, 384], bf16, kind="ExternalInput")
    wout = nc.dram_tensor("wout", [D, D], bf16, kind="ExternalInput")
    cosr = nc.dram_tensor("cosr", [128, GS], bf16, kind="ExternalInput")
    sinr = nc.dram_tensor("sinr", [128, GS], bf16, kind="ExternalInput")
    outT = nc.dram_tensor("outT", [D, LS], fp32, kind="ExternalOutput")

    Exp = mybir.ActivationFunctionType.Exp
    Copy = mybir.ActivationFunctionType.Copy
    GROUPS = [[0, 1, 2, 3, 4, 5, 6, 7]]

    with tile.TileContext(nc) as tc:
        dma = nc.default_dma_engine
        _keep = []

        def _single(*args, **kwargs):
            t, f = tc.tile(*args, **kwargs)
            _keep.append(f)
            return t

        # ---- persistent SBUF tensors ----
        wqkv_sb = _single([128, 8, 384], bf16, name="wqkv_sb")
        ident = _single([128, 128], bf16, name="ident")
        cos_sb = _single([128, GS], bf16, name="cos_sb")
        sin_sb = _single([128, GS], bf16, name="sin_sb")
        wout_sb = _single([128, 8, D], bf16, name="wout_sb")
        o1_all = _single([128, GS], bf16, name="o1_all")
        o2_all = _single([128, GS], bf16, name="o2_all")
        # per-batch head-pair tiles: rows 0:64 = head h0, 64:128 = head h1
        qp = [_single([128, S], bf16, name=f"qp{b}") for b in range(2)]
        khp = [_single([128, S], bf16, name=f"khp{b}") for b in range(2)]
        vo_all = _single([128, 32, 130], bf16, name="vo_all")
        attn_T = _single([128, 8, LS], bf16, name="attn_T")

        # ---- internal DRAM for the collective ----
        # a2aA: rank-block r = rows [128r:128(r+1)] = (my 2 heads' 128 dims)
        # x (512 q rows of block r).
        a2aA_in = _single([NC_ * 128, LS], bf16, space="DRAM", name="a2aA_in")
        a2aA_out = _single([NC_ * 128, LS], bf16, space="DRAM",
                           name="a2aA_out")

        # ---- input DMAs, in pipeline order ----
        dma.dma_start(out=wqkv_sb[:],
                      in_=wqkv[:].rearrange("(c p) k -> p c k", p=128))
        dma.dma_start(out=cos_sb[:, 0:S], in_=cosr[:, 0:S])
        dma.dma_start(out=sin_sb[:, 0:S], in_=sinr[:, 0:S])
        dma.dma_start(out=cos_sb[:, S:GS], in_=cosr[:, S:GS])
        dma.dma_start(out=sin_sb[:, S:GS], in_=sinr[:, S:GS])

        from concourse import masks as _masks
        _masks.make_identity(nc, ident[:])

        # vo_all[p, kbg, [v_h0(64) | 1 | v_h1(64) | 1]]
        nc.vector.memset(vo_all[:, :, 64:65], 1.0)
        nc.vector.memset(vo_all[:, :, 129:130], 1.0)

        xq_pool_cm = tc.tile_pool(name="xq_pool", bufs=2)
        xq_pool = xq_pool_cm.__enter__()
        # Pre-issue the first two xT quarters on the scalar-engine HWDGE
        # queue so the projection input never queues behind other sync-queue
        # transfers.
        xq_tiles = []
        for q4 in range(2):
            xq = xq_pool.tile([128, 8, 1024], bf16, tag="xq")
            nc.scalar.dma_start(
                out=xq[:],
                in_=xT[:, 1024 * q4:1024 * (q4 + 1)].rearrange(
                    "(c p) s -> p c s", p=128),
            )
            xq_tiles.append(xq)

        # ========== phase 1: QKV projection + rope + V transpose ==========
        # Per 512-seq chunk: ps1 = qk_x1 channels, ps2 = qk_x2 channels
        # (both roped), ps3 = v^T channels (PE-transposed into vo_all).
        with (
            tc.tile_pool(name="ps_kq", bufs=6, space="PSUM") as ps_kq,
            tc.tile_pool(name="ps_tr", bufs=2, space="PSUM") as ps_tr,
            tc.tile_pool(name="rope_t", bufs=4) as rope_t,
            tc.tile_pool(name="vtp", bufs=2) as vtp,
        ):
            for q4 in range(4):
                if q4 < 2:
                    xq = xq_tiles[q4]
                else:
                    xq = xq_pool.tile([128, 8, 1024], bf16, tag="xq")
                    dma.dma_start(
                        out=xq[:],
                        in_=xT[:, 1024 * q4:1024 * (q4 + 1)].rearrange(
                            "(c p) s -> p c s", p=128),
                    )
                for s2 in range(2):
                    sl = slice(1024 * q4 + 512 * s2, 1024 * q4 + 512 * (s2 + 1))
                    xsl = slice(512 * s2, 512 * (s2 + 1))
                    ps1 = ps_kq.tile([128, 512], fp32, tag="pskq")
                    for d8 in range(8):
                        nc.tensor.matmul(
                            ps1[:], wqkv_sb[:, d8, 0:128], xq[:, d8, xsl],
                            start=(d8 == 0), stop=(d8 == 7),
                        )
                    ps2 = ps_kq.tile([128, 512], fp32, tag="pskq")
                    for d8 in range(8):
                        nc.tensor.matmul(
                            ps2[:], wqkv_sb[:, d8, 128:256], xq[:, d8, xsl],
                            start=(d8 == 0), stop=(d8 == 7),
                        )
                    ps3 = ps_kq.tile([128, 512], fp32, tag="pskq")
                    for d8 in range(8):
                        nc.tensor.matmul(
                            ps3[:], wqkv_sb[:, d8, 256:384], xq[:, d8, xsl],
                            start=(d8 == 0), stop=(d8 == 7),
                        )
                    cs, sn = cos_sb[:, sl], sin_sb[:, sl]
                    t1 = rope_t.tile([128, 512], bf16, tag="rt")
                    t2 = rope_t.tile([128, 512], bf16, tag="rt")
                    nc.vector.tensor_mul(t1[:], ps1[:], cs)
                    nc.vector.tensor_mul(t2[:], ps2[:], sn)
                    nc.vector.tensor_sub(o1_all[:, sl], t1[:], t2[:])
                    nc.vector.tensor_mul(t1[:], ps1[:], sn)
                    nc.vector.tensor_mul(t2[:], ps2[:], cs)
                    nc.vector.tensor_add(o2_all[:, sl], t1[:], t2[:])
                    # v^T -> bf16 -> PE transpose -> vo_all (natural layout)
                    vt = vtp.tile([128, 512], bf16, tag="vt")
                    nc.scalar.activation(vt[:], ps3[:], Copy)
                    for t4 in range(4):
                        kbg = 8 * q4 + 4 * s2 + t4
                        ptr = ps_tr.tile([128, 128], bf16, tag="ptr")
                        nc.tensor.transpose(
                            ptr[:], vt[:, 128 * t4:128 * (t4 + 1)], ident[:])
                        nc.vector.tensor_copy(vo_all[:, kbg, 0:64],
                                              ptr[:, 0:64])
                        nc.vector.tensor_copy(vo_all[:, kbg, 65:129],
                                              ptr[:, 64:128])
                if q4 % 2 == 1:
                    # batch bb fully roped: assemble its head-pair tiles
                    # qp/khp rows: 0:32 h0_x1', 32:64 h0_x2', 64:96 h1_x1',
                    # 96:128 h1_x2'
                    bb = q4 // 2
                    bsl = slice(S * bb, S * (bb + 1))
                    for hl in range(2):
                        dma.dma_start(
                            out=qp[bb][64 * hl:64 * hl + 32, :],
                            in_=o1_all[32 * hl:32 * (hl + 1), bsl])
                        dma.dma_start(
                            out=qp[bb][64 * hl + 32:64 * hl + 64, :],
                            in_=o2_all[32 * hl:32 * (hl + 1), bsl])
                        dma.dma_start(
                            out=khp[bb][64 * hl:64 * hl + 32, :],
                            in_=o1_all[64 + 32 * hl:64 + 32 * (hl + 1), bsl])
                        dma.dma_start(
                            out=khp[bb][64 * hl + 32:64 * hl + 64, :],
                            in_=o2_all[64 + 32 * hl:64 + 32 * (hl + 1), bsl])

        xq_pool_cm.__exit__(None, None, None)

        # wout load issued here: the queue reaches it right after assembly
        # (~mid-kernel), well before the output projection needs it.
        for half in range(2):
            dma.dma_start(
                out=wout_sb[:, :, 512 * half:512 * (half + 1)],
                in_=wout[:, 512 * half:512 * (half + 1)].rearrange(
                    "(c p) v -> p c v", p=128),
            )

        if stage == 1:
            with tc.tile_pool(name="dbg", bufs=2) as dbg:
                o = dbg.tile([128, LS], fp32, name="dbgo")
                nc.vector.tensor_copy(o[:, 0:64], vo_all[:, 0, 0:64])
                nc.vector.tensor_copy(o[:, 64:128], vo_all[:, 17, 65:129])
                nc.vector.memset(o[:, 128:LS], 0.0)
                dma.dma_start(out=outT[0:128, :], in_=o[:])
                o2 = dbg.tile([128, LS], fp32, name="dbgo2")
                nc.vector.tensor_copy(o2[:], qp[0][:, 0:LS])
                dma.dma_start(out=outT[128:256, :], in_=o2[:])
                o3 = dbg.tile([128, LS], fp32, name="dbgo3")
                nc.vector.tensor_copy(o3[:], khp[0][:, 0:LS])
                dma.dma_start(out=outT[256:384, :], in_=o3[:])
            for f in reversed(_keep):
                f()
            nc.compile()
            return nc

        # ================= phase 3: attention ==============================
        with (
            tc.tile_pool(name="ps_sc", bufs=2, space="PSUM") as ps_sc,
            tc.tile_pool(name="ps_out", bufs=4, space="PSUM") as ps_out,
            tc.tile_pool(name="p_pool", bufs=32) as p_pool,
            tc.tile_pool(name="fin", bufs=4) as fin,
        ):
            for bb in range(2):
                for j in range(4):
                    qsl = slice(512 * j, 512 * (j + 1))
                    outp = [ps_out.tile([128, 512], fp32, tag="pso",
                                        name=f"outp{bb}_{j}_{u}")
                            for u in range(2)]
                    for kb in range(16):
                        sc_ps = ps_sc.tile([128, 1024], fp32, tag="sc")
                        ksl = slice(128 * kb, 128 * (kb + 1))
                        for u in range(2):
                            nc.tensor.matmul(
                                sc_ps[:, 512 * u:512 * (u + 1)],
                                khp[bb][64 * u:64 * (u + 1), ksl],
                                qp[bb][64 * u:64 * (u + 1), qsl],
                                start=True, stop=True,
                            )
                        p_sb = p_pool.tile([128, 1024], bf16, tag="p")
                        nc.scalar.activation(p_sb[:], sc_ps[:], Exp,
                                             scale=0.125)
                        for u in range(2):
                            nc.tensor.matmul(
                                outp[u][0:65, :],
                                vo_all[:, 16 * bb + kb, 65 * u:65 * (u + 1)],
                                p_sb[:, 512 * u:512 * (u + 1)],
                                start=(kb == 0), stop=(kb == 15),
                                skip_group_check=True,
                            )
                    for u in range(2):
                        recip = fin.tile([1, 512], fp32, tag="recip")
                        nc.vector.reciprocal(recip[:], outp[u][64:65, :])
                        bcast = fin.tile([64, 512], fp32, tag="bcast")
                        nc.gpsimd.partition_broadcast(bcast[:], recip[:])
                        att = fin.tile([64, 512], bf16, tag="att")
                        nc.vector.tensor_mul(att[:], outp[u][0:64, :],
                                             bcast[:])
                        row0 = 128 * (4 * bb + j) + 64 * u
                        dma.dma_start(
                            out=a2aA_in[row0:row0 + 64, :],
                            in_=att[:],
                        )

        nc.gpsimd.collective_compute(
            "AllToAll",
            mybir.AluOpType.bypass,
            replica_groups=GROUPS,
            ins=[a2aA_in[:].opt()],
            outs=[a2aA_out[:].opt()],
        )
        for t in range(8):
            dma.dma_start(out=attn_T[:, t, :],
                          in_=a2aA_out[128 * t:128 * (t + 1), :])

        # ================= phase 4: output projection ======================
        with (
            tc.tile_pool(name="ps_o", bufs=4, space="PSUM") as ps_o,
            tc.tile_pool(name="ocp", bufs=4) as ocp,
        ):
            for oc in range(8):
                ps = ps_o.tile([128, 512], fp32, tag="pso2")
                for t in range(8):
                    nc.tensor.matmul(
                        ps[:], wout_sb[:, t, 128 * oc:128 * (oc + 1)],
                        attn_T[:, t, :],
                        start=(t == 0), stop=(t == 7),
                    )
                ot = ocp.tile([128, 512], fp32, tag="ocp")
                nc.scalar.activation(ot[:], ps[:], Copy)
                dma.dma_start(out=outT[128 * oc:128 * (oc + 1), :], in_=ot[:])

        for f in reversed(_keep):
            f()

    nc.compile()
    return nc


def _host_prep(inputs, positions, w_in, w_out):
    inputs = np.asarray(inputs, np.float32)
    positions = np.asarray(positions)
    w_in = np.asarray(w_in, np.float32)
    w_out = np.asarray(w_out, np.float32)

    x_all = np.concatenate([inputs[0], inputs[1]], axis=0)          # (4096, D)
    xT_full = np.ascontiguousarray(x_all.T).astype(BF16)            # (D, 4096)

    ar32, ar64 = np.arange(32), np.arange(64)
    wout_b = np.ascontiguousarray(w_out).astype(BF16)

    inv_freq = 1.0 / (ROPE_THETA ** (np.arange(32, dtype=np.float32) / 32))
    pos_all = np.concatenate([positions[0], positions[1]]).astype(np.float32)
    ang = pos_all[None, :] * inv_freq[:, None]                      # (32, 4096)
    cosr = np.ascontiguousarray(np.tile(np.cos(ang), (4, 1))).astype(BF16)
    sinr = np.ascontiguousarray(np.tile(np.sin(ang), (4, 1))).astype(BF16)

    in_maps = []
    for c in range(NC_):
        H0, H1 = 2 * c, 2 * c + 1
        cols = np.concatenate([
            192 * H0 + ar32, 192 * H1 + ar32,            # q_x1 h0, h1
            192 * H0 + 64 + ar32, 192 * H1 + 64 + ar32,  # k_x1 h0, h1
            192 * H0 + 32 + ar32, 192 * H1 + 32 + ar32,  # q_x2 h0, h1
            192 * H0 + 96 + ar32, 192 * H1 + 96 + ar32,  # k_x2 h0, h1
            192 * H0 + 128 + ar64, 192 * H1 + 128 + ar64,  # v h0, h1
        ])
        wqkv = np.ascontiguousarray(w_in[:, cols]).astype(BF16)
        in_maps.append({
            "xT": xT_full, "wqkv": wqkv,
            "wout": wout_b, "cosr": cosr, "sinr": sinr,
        })
    return in_maps


def kernel(inputs, positions, w_in, w_out, _trace=False):
    global _COMPILED
    from concourse.bass_utils import run_bass_kernel_spmd

    if _COMPILED is None:
        _COMPILED = _build()
    nc = _COMPILED

    in_maps = _host_prep(inputs, positions, w_in, w_out)
    res = run_bass_kernel_spmd(
        nc, in_maps, core_ids=list(range(NC_)), trace=_trace
    )
    kernel.last_results = res

    out = np.zeros((B, S, D), np.float32)
    for c in range(NC_):
        b, j = divmod(c, 4)
        out[b, LS * j:LS * (j + 1), :] = res.results[c]["outT"].T
    return out


# revision 40
# speedup vs baseline: 3.1412x; 1.2005x over previous
"""Distributed Trainium2 kernel for the AttentionBlock problem.

Sharding (v2): tensor-parallel over heads for QKV+attention (each of the 8
cores owns 2 heads for both batches), sequence-parallel for the V projection
and the output projection (each core owns one 512-row block of the flattened
(B*S) dimension).  Two small (1 MB) AllToAll collectives glue the layouts
together:

  1. V is projected seq-parallel (wide, efficient matmuls), then AllToAll'd
     so every core holds V for its own 2 heads over all 4096 rows.  This
     collective overlaps the Q/K projection + RoPE.
  2. After attention, normalized head outputs are AllToAll'd so every core
     holds all 1024 head-dims for its own 512 rows, then applies the full
     output projection locally.  Outputs concatenate on the host.

Device notes:
- All matmul inputs are bf16, PSUM accumulates f32.
- Attention scores are computed transposed (k on partitions, q free) so the
  softmax exp feeds the PV matmul directly; the softmax denominator comes
  from a leading all-ones column prepended to each head's V block.
- Score matmuls for the two batches of a head are row-tiled into the PE
  array concurrently (each uses 64 of the 128 contraction rows).
- The exp runs on the scalar engine at [128,1024] per score block; with
  2-deep score PSUM and 4-deep output PSUM this fits exactly in 8 banks.
"""

import numpy as np
import ml_dtypes

BF16 = ml_dtypes.bfloat16
H, HD, D, B, S = 16, 64, 1024, 2, 2048
LS = 512            # seq rows per core for V / out projection
NC_ = 8
GS = B * S          # 4096 flattened rows
ROPE_THETA = 10000.0

_COMPILED = None


def _build(stage=3):
    import concourse.bass as bass
    import concourse.mybir as mybir
    import concourse.tile as tile
    from concourse import bacc

    fp32 = mybir.dt.float32
    bf16 = mybir.dt.bfloat16

    nc = bacc.Bacc(
        "TRN2", target_bir_lowering=False, debug=False, num_devices=NC_
    )

    xT = nc.dram_tensor("xT", [D, GS], bf16, kind="ExternalInput")
    wqkv = nc.dram_tensor("wqkv", [D, 384], bf16, kind="ExternalInput")
    woutl = nc.dram_tensor("woutl", [128, D], bf16, kind="ExternalInput")
    cosr = nc.dram_tensor("cosr", [128, GS], bf16, kind="ExternalInput")
    sinr = nc.dram_tensor("sinr", [128, GS], bf16, kind="ExternalInput")
    # per-core partial of the output projection, transposed: rows = out
    # dims, cols = global (b*S + s) rows; host sums the 8 partials.
    outT = nc.dram_tensor("outT", [D, GS], bf16, kind="ExternalOutput")

    Exp = mybir.ActivationFunctionType.Exp
    Copy = mybir.ActivationFunctionType.Copy
    Recip = mybir.ActivationFunctionType.Reciprocal

    with tile.TileContext(nc) as tc:
        dma = nc.default_dma_engine
        _keep = []

        def _single(*args, **kwargs):
            t, f = tc.tile(*args, **kwargs)
            _keep.append(f)
            return t

        # ---- persistent SBUF tensors ----
        wqkv_sb = _single([128, 8, 384], bf16, name="wqkv_sb")
        ident = _single([128, 128], bf16, name="ident")
        cos_sb = _single([128, GS], bf16, name="cos_sb")
        sin_sb = _single([128, GS], bf16, name="sin_sb")
        wout_sb = _single([128, D], bf16, name="wout_sb")
        o1_all = _single([128, GS], bf16, name="o1_all")
        o2_all = _single([128, GS], bf16, name="o2_all")
        # per-batch head-pair tiles: rows 0:64 = head h0, 64:128 = head h1
        qp = [_single([128, S], bf16, name=f"qp{b}") for b in range(2)]
        khp = [_single([128, S], bf16, name=f"khp{b}") for b in range(2)]
        vo_all = _single([128, 32, 130], bf16, name="vo_all")
        # normalized attention outputs: [my 128 head dims, slot = 4b+j, q]
        o_all = _single([128, 8, LS], bf16, name="o_all")

        # ---- input DMAs, in pipeline order ----
        xq_pool_cm = tc.tile_pool(name="xq_pool", bufs=2)
        xq_pool = xq_pool_cm.__enter__()
        xq_tiles = []
        for q4 in range(2):
            xq = xq_pool.tile([128, 8, 1024], bf16, tag="xq")
            dma.dma_start(
                out=xq[:],
                in_=xT[:, 1024 * q4:1024 * (q4 + 1)].rearrange(
                    "(c p) s -> p c s", p=128),
            )
            xq_tiles.append(xq)
        dma.dma_start(out=wqkv_sb[:],
                      in_=wqkv[:].rearrange("(c p) k -> p c k", p=128))
        dma.dma_start(out=cos_sb[:, 0:S], in_=cosr[:, 0:S])
        dma.dma_start(out=sin_sb[:, 0:S], in_=sinr[:, 0:S])
        dma.dma_start(out=cos_sb[:, S:GS], in_=cosr[:, S:GS])
        dma.dma_start(out=sin_sb[:, S:GS], in_=sinr[:, S:GS])
        dma.dma_start(out=wout_sb[:], in_=woutl[:])

        from concourse import masks as _masks
        _masks.make_identity(nc, ident[:])

        # vo_all[p, kbg, [v_h0(64) | 1 | v_h1(64) | 1]]
        nc.vector.memset(vo_all[:, :, 64:65], 1.0)
        nc.vector.memset(vo_all[:, :, 129:130], 1.0)

        # ========== phase 1: QKV projection + rope + V transpose ==========
        # Per 512-seq chunk: ps1 = qk_x1 channels, ps2 = qk_x2 channels
        # (both roped), ps3 = v^T channels (PE-transposed into vo_all).
        with (
            tc.tile_pool(name="ps_kq", bufs=6, space="PSUM") as ps_kq,
            tc.tile_pool(name="ps_tr", bufs=2, space="PSUM") as ps_tr,
            tc.tile_pool(name="rope_t", bufs=4) as rope_t,
            tc.tile_pool(name="vtp", bufs=2) as vtp,
        ):
            for q4 in range(4):
                if q4 < 2:
                    xq = xq_tiles[q4]
                else:
                    xq = xq_pool.tile([128, 8, 1024], bf16, tag="xq")
                    dma.dma_start(
                        out=xq[:],
                        in_=xT[:, 1024 * q4:1024 * (q4 + 1)].rearrange(
                            "(c p) s -> p c s", p=128),
                    )
                for s2 in range(2):
                    sl = slice(1024 * q4 + 512 * s2, 1024 * q4 + 512 * (s2 + 1))
                    xsl = slice(512 * s2, 512 * (s2 + 1))
                    ps1 = ps_kq.tile([128, 512], fp32, tag="pskq")
                    for d8 in range(8):
                        nc.tensor.matmul(
                            ps1[:], wqkv_sb[:, d8, 0:128], xq[:, d8, xsl],
                            start=(d8 == 0), stop=(d8 == 7),
                        )
                    ps2 = ps_kq.tile([128, 512], fp32, tag="pskq")
                    for d8 in range(8):
                        nc.tensor.matmul(
                            ps2[:], wqkv_sb[:, d8, 128:256], xq[:, d8, xsl],
                            start=(d8 == 0), stop=(d8 == 7),
                        )
                    ps3 = ps_kq.tile([128, 512], fp32, tag="pskq")
                    for d8 in range(8):
                        nc.tensor.matmul(
                            ps3[:], wqkv_sb[:, d8, 256:384], xq[:, d8, xsl],
                            start=(d8 == 0), stop=(d8 == 7),
                        )
                    cs, sn = cos_sb[:, sl], sin_sb[:, sl]
                    t1 = rope_t.tile([128, 512], bf16, tag="rt")
                    t2 = rope_t.tile([128, 512], bf16, tag="rt")
                    nc.vector.tensor_mul(t1[:], ps1[:], cs)
                    nc.vector.tensor_mul(t2[:], ps2[:], sn)
                    nc.vector.tensor_sub(o1_all[:, sl], t1[:], t2[:])
                    nc.vector.tensor_mul(t1[:], ps1[:], sn)
                    nc.vector.tensor_mul(t2[:], ps2[:], cs)
                    nc.vector.tensor_add(o2_all[:, sl], t1[:], t2[:])
                    # v^T -> bf16 -> PE transpose -> vo_all (natural layout)
                    vt = vtp.tile([128, 512], bf16, tag="vt")
                    nc.vector.tensor_copy(vt[:], ps3[:])
                    for t4 in range(4):
                        kbg = 8 * q4 + 4 * s2 + t4
                        ptr = ps_tr.tile([128, 128], bf16, tag="ptr")
                        nc.tensor.transpose(
                            ptr[:], vt[:, 128 * t4:128 * (t4 + 1)], ident[:])
                        nc.vector.tensor_copy(vo_all[:, kbg, 0:64],
                                              ptr[:, 0:64])
                        nc.vector.tensor_copy(vo_all[:, kbg, 65:129],
                                              ptr[:, 64:128])
                if q4 % 2 == 1:
                    # batch bb fully roped: assemble its head-pair tiles
                    # qp/khp rows: 0:32 h0_x1', 32:64 h0_x2', 64:96 h1_x1',
                    # 96:128 h1_x2'
                    bb = q4 // 2
                    bsl = slice(S * bb, S * (bb + 1))
                    for hl in range(2):
                        dma.dma_start(
                            out=qp[bb][64 * hl:64 * hl + 32, :],
                            in_=o1_all[32 * hl:32 * (hl + 1), bsl])
                        dma.dma_start(
                            out=qp[bb][64 * hl + 32:64 * hl + 64, :],
                            in_=o2_all[32 * hl:32 * (hl + 1), bsl])
                        dma.dma_start(
                            out=khp[bb][64 * hl:64 * hl + 32, :],
                            in_=o1_all[64 + 32 * hl:64 + 32 * (hl + 1), bsl])
                        dma.dma_start(
                            out=khp[bb][64 * hl + 32:64 * hl + 64, :],
                            in_=o2_all[64 + 32 * hl:64 + 32 * (hl + 1), bsl])

        xq_pool_cm.__exit__(None, None, None)

        if stage == 1:
            with tc.tile_pool(name="dbg", bufs=2) as dbg:
                o = dbg.tile([128, LS], fp32, name="dbgo")
                nc.vector.tensor_copy(o[:, 0:64], vo_all[:, 0, 0:64])
                nc.vector.tensor_copy(o[:, 64:128], vo_all[:, 17, 65:129])
                nc.vector.memset(o[:, 128:LS], 0.0)
                dma.dma_start(out=outT[0:128, :], in_=o[:])
                o2 = dbg.tile([128, LS], fp32, name="dbgo2")
                nc.vector.tensor_copy(o2[:], qp[0][:, 0:LS])
                dma.dma_start(out=outT[128:256, :], in_=o2[:])
                o3 = dbg.tile([128, LS], fp32, name="dbgo3")
                nc.vector.tensor_copy(o3[:], khp[0][:, 0:LS])
                dma.dma_start(out=outT[256:384, :], in_=o3[:])
            for f in reversed(_keep):
                f()
            nc.compile()
            return nc

        # ================= phase 3: attention ==============================
        with (
            tc.tile_pool(name="ps_sc", bufs=2, space="PSUM") as ps_sc,
            tc.tile_pool(name="ps_out", bufs=4, space="PSUM") as ps_out,
            tc.tile_pool(name="p_pool", bufs=32) as p_pool,
            tc.tile_pool(name="fin", bufs=4) as fin,
        ):
            for bb in range(2):
                for j in range(4):
                    qsl = slice(512 * j, 512 * (j + 1))
                    outp = [ps_out.tile([128, 512], fp32, tag="pso",
                                        name=f"outp{bb}_{j}_{u}")
                            for u in range(2)]
                    for kb in range(16):
                        sc_ps = ps_sc.tile([128, 1024], fp32, tag="sc")
                        ksl = slice(128 * kb, 128 * (kb + 1))
                        for u in range(2):
                            nc.tensor.matmul(
                                sc_ps[:, 512 * u:512 * (u + 1)],
                                khp[bb][64 * u:64 * (u + 1), ksl],
                                qp[bb][64 * u:64 * (u + 1), qsl],
                                start=True, stop=True,
                            )
                        p_sb = p_pool.tile([128, 1024], bf16, tag="p")
                        nc.scalar.activation(p_sb[:], sc_ps[:], Exp,
                                             scale=0.125)
                        for u in range(2):
                            nc.tensor.matmul(
                                outp[u][0:65, :],
                                vo_all[:, 16 * bb + kb, 65 * u:65 * (u + 1)],
                                p_sb[:, 512 * u:512 * (u + 1)],
                                start=(kb == 0), stop=(kb == 15),
                                skip_group_check=True,
                            )
                    slot = 4 * bb + j
                    for u in range(2):
                        recip = fin.tile([1, 512], fp32, tag="recip")
                        nc.vector.reciprocal(recip[:], outp[u][64:65, :])
                        bcast = fin.tile([64, 512], fp32, tag="bcast")
                        nc.gpsimd.partition_broadcast(bcast[:], recip[:])
                        nc.vector.tensor_mul(
                            o_all[64 * u:64 * (u + 1), slot, :],
                            outp[u][0:64, :], bcast[:])

        # ====== phase 3: partial output projection (my 128 head dims) =====
        with (
            tc.tile_pool(name="ps_o", bufs=4, space="PSUM") as ps_o,
            tc.tile_pool(name="ocp", bufs=4) as ocp,
        ):
            for slot in range(8):
                for oc in range(8):
                    ps = ps_o.tile([128, 512], fp32, tag="pso2")
                    nc.tensor.matmul(
                        ps[:], wout_sb[:, 128 * oc:128 * (oc + 1)],
                        o_all[:, slot, :],
                        start=True, stop=True,
                    )
                    ot = ocp.tile([128, 512], bf16, tag="ocp")
                    nc.scalar.activation(ot[:], ps[:], Copy)
                    dma.dma_start(
                        out=outT[128 * oc:128 * (oc + 1),
                                 512 * slot:512 * (slot + 1)],
                        in_=ot[:])

        for f in reversed(_keep):
            f()

    nc.compile()
    return nc


def _host_prep(inputs, positions, w_in, w_out):
    inputs = np.asarray(inputs, np.float32)
    positions = np.asarray(positions)
    w_in = np.asarray(w_in, np.float32)
    w_out = np.asarray(w_out, np.float32)

    x_all = np.concatenate([inputs[0], inputs[1]], axis=0)          # (4096, D)
    xT_full = np.ascontiguousarray(x_all.T).astype(BF16)            # (D, 4096)

    ar32, ar64 = np.arange(32), np.arange(64)

    inv_freq = 1.0 / (ROPE_THETA ** (np.arange(32, dtype=np.float32) / 32))
    pos_all = np.concatenate([positions[0], positions[1]]).astype(np.float32)
    ang = pos_all[None, :] * inv_freq[:, None]                      # (32, 4096)
    cosr = np.ascontiguousarray(np.tile(np.cos(ang), (4, 1))).astype(BF16)
    sinr = np.ascontiguousarray(np.tile(np.sin(ang), (4, 1))).astype(BF16)

    in_maps = []
    for c in range(NC_):
        H0, H1 = 2 * c, 2 * c + 1
        cols = np.concatenate([
            192 * H0 + ar32, 192 * H1 + ar32,            # q_x1 h0, h1
            192 * H0 + 64 + ar32, 192 * H1 + 64 + ar32,  # k_x1 h0, h1
            192 * H0 + 32 + ar32, 192 * H1 + 32 + ar32,  # q_x2 h0, h1
            192 * H0 + 96 + ar32, 192 * H1 + 96 + ar32,  # k_x2 h0, h1
            192 * H0 + 128 + ar64, 192 * H1 + 128 + ar64,  # v h0, h1
        ])
        wqkv = np.ascontiguousarray(w_in[:, cols]).astype(BF16)
        # rows of w_out for my two heads' output dims
        woutl = np.ascontiguousarray(
            w_out[128 * c:128 * (c + 1), :]).astype(BF16)
        in_maps.append({
            "xT": xT_full, "wqkv": wqkv,
            "woutl": woutl, "cosr": cosr, "sinr": sinr,
        })
    return in_maps


def kernel(inputs, positions, w_in, w_out, _trace=False):
    global _COMPILED
    from concourse.bass_utils import run_bass_kernel_spmd

    if _COMPILED is None:
        _COMPILED = _build()
    nc = _COMPILED

    in_maps = _host_prep(inputs, positions, w_in, w_out)
    res = run_bass_kernel_spmd(
        nc, in_maps, core_ids=list(range(NC_)), trace=_trace
    )
    kernel.last_results = res

    acc = np.zeros((D, GS), np.float32)
    for c in range(NC_):
        acc += np.asarray(res.results[c]["outT"], dtype=np.float32)
    return np.ascontiguousarray(acc.T).reshape(B, S, D)


# revision 43
# speedup vs baseline: 3.2017x; 1.0192x over previous
"""Distributed Trainium2 kernel for the AttentionBlock problem.

Sharding (v2): tensor-parallel over heads for QKV+attention (each of the 8
cores owns 2 heads for both batches), sequence-parallel for the V projection
and the output projection (each core owns one 512-row block of the flattened
(B*S) dimension).  Two small (1 MB) AllToAll collectives glue the layouts
together:

  1. V is projected seq-parallel (wide, efficient matmuls), then AllToAll'd
     so every core holds V for its own 2 heads over all 4096 rows.  This
     collective overlaps the Q/K projection + RoPE.
  2. After attention, normalized head outputs are AllToAll'd so every core
     holds all 1024 head-dims for its own 512 rows, then applies the full
     output projection locally.  Outputs concatenate on the host.

Device notes:
- All matmul inputs are bf16, PSUM accumulates f32.
- Attention scores are computed transposed (k on partitions, q free) so the
  softmax exp feeds the PV matmul directly; the softmax denominator comes
  from a leading all-ones column prepended to each head's V block.
- Score matmuls for the two batches of a head are row-tiled into the PE
  array concurrently (each uses 64 of the 128 contraction rows).
- The exp runs on the scalar engine at [128,1024] per score block; with
  2-deep score PSUM and 4-deep output PSUM this fits exactly in 8 banks.
"""

import numpy as np
import ml_dtypes

BF16 = ml_dtypes.bfloat16
H, HD, D, B, S = 16, 64, 1024, 2, 2048
LS = 512            # seq rows per core for V / out projection
NC_ = 8
GS = B * S          # 4096 flattened rows
ROPE_THETA = 10000.0

_COMPILED = None


def _build(stage=3):
    import concourse.bass as bass
    import concourse.mybir as mybir
    import concourse.tile as tile
    from concourse import bacc

    fp32 = mybir.dt.float32
    bf16 = mybir.dt.bfloat16

    nc = bacc.Bacc(
        "TRN2", target_bir_lowering=False, debug=False, num_devices=NC_
    )

    xT = nc.dram_tensor("xT", [D, GS], bf16, kind="ExternalInput")
    wqkv = nc.dram_tensor("wqkv", [D, 384], bf16, kind="ExternalInput")
    woutl = nc.dram_tensor("woutl", [128, D], bf16, kind="ExternalInput")
    cosr = nc.dram_tensor("cosr", [128, GS], bf16, kind="ExternalInput")
    sinr = nc.dram_tensor("sinr", [128, GS], bf16, kind="ExternalInput")
    # per-core partial of the output projection, transposed: rows = out
    # dims, cols = global (b*S + s) rows; host sums the 8 partials.
    outT = nc.dram_tensor("outT", [D, GS], bf16, kind="ExternalOutput")

    Exp = mybir.ActivationFunctionType.Exp
    Copy = mybir.ActivationFunctionType.Copy
    Recip = mybir.ActivationFunctionType.Reciprocal

    with tile.TileContext(nc) as tc:
        dma = nc.default_dma_engine
        _keep = []

        def _single(*args, **kwargs):
            t, f = tc.tile(*args, **kwargs)
            _keep.append(f)
            return t

        # ---- persistent SBUF tensors ----
        wqkv_sb = _single([128, 8, 384], bf16, name="wqkv_sb")
        ident = _single([128, 128], bf16, name="ident")
        cos_sb = _single([128, GS], bf16, name="cos_sb")
        sin_sb = _single([128, GS], bf16, name="sin_sb")
        wout_sb = _single([128, D], bf16, name="wout_sb")
        o1_all = _single([128, GS], bf16, name="o1_all")
        o2_all = _single([128, GS], bf16, name="o2_all")
        # per-batch head-pair tiles: rows 0:64 = head h0, 64:128 = head h1
        qp = [_single([128, S], bf16, name=f"qp{b}") for b in range(2)]
        khp = [_single([128, S], bf16, name=f"khp{b}") for b in range(2)]
        vo_all = _single([128, 32, 130], bf16, name="vo_all")
        # normalized attention outputs: [my 128 head dims, slot = 4b+j, q]
        o_all = _single([128, 8, LS], bf16, name="o_all")

        # ---- input DMAs, in pipeline order ----
        dma.dma_start(out=wqkv_sb[:],
                      in_=wqkv[:].rearrange("(c p) k -> p c k", p=128))
        xq_pool_cm = tc.tile_pool(name="xq_pool", bufs=2)
        xq_pool = xq_pool_cm.__enter__()
        xq_tiles = []
        for q4 in range(2):
            xq = xq_pool.tile([128, 8, 1024], bf16, tag="xq")
            dma.dma_start(
                out=xq[:],
                in_=xT[:, 1024 * q4:1024 * (q4 + 1)].rearrange(
                    "(c p) s -> p c s", p=128),
            )
            xq_tiles.append(xq)
        dma.dma_start(out=cos_sb[:, 0:S], in_=cosr[:, 0:S])
        dma.dma_start(out=sin_sb[:, 0:S], in_=sinr[:, 0:S])
        dma.dma_start(out=cos_sb[:, S:GS], in_=cosr[:, S:GS])
        dma.dma_start(out=sin_sb[:, S:GS], in_=sinr[:, S:GS])
        dma.dma_start(out=wout_sb[:], in_=woutl[:])

        from concourse import masks as _masks
        _masks.make_identity(nc, ident[:])

        # vo_all[p, kbg, [v_h0(64) | 1 | v_h1(64) | 1]]
        nc.vector.memset(vo_all[:, :, 64:65], 1.0)
        nc.vector.memset(vo_all[:, :, 129:130], 1.0)

        # ========== phase 1: QKV projection + rope + V transpose ==========
        # Per 512-seq chunk: ps1 = qk_x1 channels, ps2 = qk_x2 channels
        # (both roped), ps3 = v^T channels (PE-transposed into vo_all).
        with (
            tc.tile_pool(name="ps_kq", bufs=6, space="PSUM") as ps_kq,
            tc.tile_pool(name="ps_tr", bufs=2, space="PSUM") as ps_tr,
            tc.tile_pool(name="rope_t", bufs=4) as rope_t,
            tc.tile_pool(name="vtp", bufs=2) as vtp,
        ):
            for q4 in range(4):
                if q4 < 2:
                    xq = xq_tiles[q4]
                else:
                    xq = xq_pool.tile([128, 8, 1024], bf16, tag="xq")
                    dma.dma_start(
                        out=xq[:],
                        in_=xT[:, 1024 * q4:1024 * (q4 + 1)].rearrange(
                            "(c p) s -> p c s", p=128),
                    )
                for s2 in range(2):
                    sl = slice(1024 * q4 + 512 * s2, 1024 * q4 + 512 * (s2 + 1))
                    xsl = slice(512 * s2, 512 * (s2 + 1))
                    ps1 = ps_kq.tile([128, 512], fp32, tag="pskq")
                    for d8 in range(8):
                        nc.tensor.matmul(
                            ps1[:], wqkv_sb[:, d8, 0:128], xq[:, d8, xsl],
                            start=(d8 == 0), stop=(d8 == 7),
                        )
                    ps2 = ps_kq.tile([128, 512], fp32, tag="pskq")
                    for d8 in range(8):
                        nc.tensor.matmul(
                            ps2[:], wqkv_sb[:, d8, 128:256], xq[:, d8, xsl],
                            start=(d8 == 0), stop=(d8 == 7),
                        )
                    ps3 = ps_kq.tile([128, 512], fp32, tag="pskq")
                    for d8 in range(8):
                        nc.tensor.matmul(
                            ps3[:], wqkv_sb[:, d8, 256:384], xq[:, d8, xsl],
                            start=(d8 == 0), stop=(d8 == 7),
                        )
                    cs, sn = cos_sb[:, sl], sin_sb[:, sl]
                    t1 = rope_t.tile([128, 512], bf16, tag="rt")
                    t2 = rope_t.tile([128, 512], bf16, tag="rt")
                    nc.vector.tensor_mul(t1[:], ps1[:], cs)
                    nc.vector.tensor_mul(t2[:], ps2[:], sn)
                    nc.vector.tensor_sub(o1_all[:, sl], t1[:], t2[:])
                    nc.vector.tensor_mul(t1[:], ps1[:], sn)
                    nc.vector.tensor_mul(t2[:], ps2[:], cs)
                    nc.vector.tensor_add(o2_all[:, sl], t1[:], t2[:])
                    # v^T -> bf16 -> PE transpose -> vo_all (natural layout)
                    vt = vtp.tile([128, 512], bf16, tag="vt")
                    nc.vector.tensor_copy(vt[:], ps3[:])
                    for t4 in range(4):
                        kbg = 8 * q4 + 4 * s2 + t4
                        ptr = ps_tr.tile([128, 128], bf16, tag="ptr")
                        nc.tensor.transpose(
                            ptr[:], vt[:, 128 * t4:128 * (t4 + 1)], ident[:])
                        nc.vector.tensor_copy(vo_all[:, kbg, 0:64],
                                              ptr[:, 0:64])
                        nc.vector.tensor_copy(vo_all[:, kbg, 65:129],
                                              ptr[:, 64:128])
                if q4 % 2 == 1:
                    # batch bb fully roped: assemble its head-pair tiles
                    # qp/khp rows: 0:32 h0_x1', 32:64 h0_x2', 64:96 h1_x1',
                    # 96:128 h1_x2'
                    bb = q4 // 2
                    bsl = slice(S * bb, S * (bb + 1))
                    for hl in range(2):
                        dma.dma_start(
                            out=qp[bb][64 * hl:64 * hl + 32, :],
                            in_=o1_all[32 * hl:32 * (hl + 1), bsl])
                        dma.dma_start(
                            out=qp[bb][64 * hl + 32:64 * hl + 64, :],
                            in_=o2_all[32 * hl:32 * (hl + 1), bsl])
                        dma.dma_start(
                            out=khp[bb][64 * hl:64 * hl + 32, :],
                            in_=o1_all[64 + 32 * hl:64 + 32 * (hl + 1), bsl])
                        dma.dma_start(
                            out=khp[bb][64 * hl + 32:64 * hl + 64, :],
                            in_=o2_all[64 + 32 * hl:64 + 32 * (hl + 1), bsl])

        xq_pool_cm.__exit__(None, None, None)

        if stage == 1:
            with tc.tile_pool(name="dbg", bufs=2) as dbg:
                o = dbg.tile([128, LS], fp32, name="dbgo")
                nc.vector.tensor_copy(o[:, 0:64], vo_all[:, 0, 0:64])
                nc.vector.tensor_copy(o[:, 64:128], vo_all[:, 17, 65:129])
                nc.vector.memset(o[:, 128:LS], 0.0)
                dma.dma_start(out=outT[0:128, :], in_=o[:])
                o2 = dbg.tile([128, LS], fp32, name="dbgo2")
                nc.vector.tensor_copy(o2[:], qp[0][:, 0:LS])
                dma.dma_start(out=outT[128:256, :], in_=o2[:])
                o3 = dbg.tile([128, LS], fp32, name="dbgo3")
                nc.vector.tensor_copy(o3[:], khp[0][:, 0:LS])
                dma.dma_start(out=outT[256:384, :], in_=o3[:])
            for f in reversed(_keep):
                f()
            nc.compile()
            return nc

        # ================= phase 3: attention ==============================
        with (
            tc.tile_pool(name="ps_sc", bufs=2, space="PSUM") as ps_sc,
            tc.tile_pool(name="ps_out", bufs=4, space="PSUM") as ps_out,
            tc.tile_pool(name="p_pool", bufs=32) as p_pool,
            tc.tile_pool(name="fin", bufs=4) as fin,
            tc.tile_pool(name="ocp", bufs=4) as ocp,
        ):
            def emit_job(bb, j):
                qsl = slice(512 * j, 512 * (j + 1))
                outp = [ps_out.tile([128, 512], fp32, tag="pso",
                                    name=f"outp{bb}_{j}_{u}")
                        for u in range(2)]
                for kb in range(16):
                    sc_ps = ps_sc.tile([128, 1024], fp32, tag="sc")
                    ksl = slice(128 * kb, 128 * (kb + 1))
                    for u in range(2):
                        nc.tensor.matmul(
                            sc_ps[:, 512 * u:512 * (u + 1)],
                            khp[bb][64 * u:64 * (u + 1), ksl],
                            qp[bb][64 * u:64 * (u + 1), qsl],
                            start=True, stop=True,
                        )
                    p_sb = p_pool.tile([128, 1024], bf16, tag="p")
                    nc.scalar.activation(p_sb[:], sc_ps[:], Exp, scale=0.125)
                    for u in range(2):
                        nc.tensor.matmul(
                            outp[u][0:65, :],
                            vo_all[:, 16 * bb + kb, 65 * u:65 * (u + 1)],
                            p_sb[:, 512 * u:512 * (u + 1)],
                            start=(kb == 0), stop=(kb == 15),
                            skip_group_check=True,
                        )
                slot = 4 * bb + j
                for u in range(2):
                    recip = fin.tile([1, 512], fp32, tag="recip")
                    nc.vector.reciprocal(recip[:], outp[u][64:65, :])
                    bcast = fin.tile([64, 512], fp32, tag="bcast")
                    nc.gpsimd.partition_broadcast(bcast[:], recip[:])
                    nc.vector.tensor_mul(
                        o_all[64 * u:64 * (u + 1), slot, :],
                        outp[u][0:64, :], bcast[:])

            def emit_oproj(slot):
                # partial output projection for one 512-row slot, sharing
                # the ps_out pool; copies on DVE so the exp stream on the
                # scalar engine is never interrupted.
                for oc in range(8):
                    ps = ps_out.tile([128, 512], fp32, tag="pso")
                    nc.tensor.matmul(
                        ps[:], wout_sb[:, 128 * oc:128 * (oc + 1)],
                        o_all[:, slot, :],
                        start=True, stop=True,
                    )
                    ot = ocp.tile([128, 512], bf16, tag="ocp")
                    nc.vector.tensor_copy(ot[:], ps[:])
                    dma.dma_start(
                        out=outT[128 * oc:128 * (oc + 1),
                                 512 * slot:512 * (slot + 1)],
                        in_=ot[:])

            sched = [(0, 0, None), (0, 1, None), (0, 2, 0), (0, 3, 1),
                     (1, 0, 2), (1, 1, 3), (1, 2, 4), (1, 3, 5)]
            for bb, j, opr in sched:
                emit_job(bb, j)
                if opr is not None:
                    emit_oproj(opr)
            emit_oproj(6)
            emit_oproj(7)

        for f in reversed(_keep):
            f()

    nc.compile()
    return nc


def _host_prep(inputs, positions, w_in, w_out):
    inputs = np.asarray(inputs, np.float32)
    positions = np.asarray(positions)
    w_in = np.asarray(w_in, np.float32)
    w_out = np.asarray(w_out, np.float32)

    x_all = np.concatenate([inputs[0], inputs[1]], axis=0)          # (4096, D)
    xT_full = np.ascontiguousarray(x_all.T).astype(BF16)            # (D, 4096)

    ar32, ar64 = np.arange(32), np.arange(64)

    inv_freq = 1.0 / (ROPE_THETA ** (np.arange(32, dtype=np.float32) / 32))
    pos_all = np.concatenate([positions[0], positions[1]]).astype(np.float32)
    ang = pos_all[None, :] * inv_freq[:, None]                      # (32, 4096)
    cosr = np.ascontiguousarray(np.tile(np.cos(ang), (4, 1))).astype(BF16)
    sinr = np.ascontiguousarray(np.tile(np.sin(ang), (4, 1))).astype(BF16)

    in_maps = []
    for c in range(NC_):
        H0, H1 = 2 * c, 2 * c + 1
        cols = np.concatenate([
            192 * H0 + ar32, 192 * H1 + ar32,            # q_x1 h0, h1
            192 * H0 + 64 + ar32, 192 * H1 + 64 + ar32,  # k_x1 h0, h1
            192 * H0 + 32 + ar32, 192 * H1 + 32 + ar32,  # q_x2 h0, h1
            192 * H0 + 96 + ar32, 192 * H1 + 96 + ar32,  # k_x2 h0, h1
            192 * H0 + 128 + ar64, 192 * H1 + 128 + ar64,  # v h0, h1
        ])
        wqkv = np.ascontiguousarray(w_in[:, cols]).astype(BF16)
        # rows of w_out for my two heads' output dims
        woutl = np.ascontiguousarray(
            w_out[128 * c:128 * (c + 1), :]).astype(BF16)
        in_maps.append({
            "xT": xT_full, "wqkv": wqkv,
            "woutl": woutl, "cosr": cosr, "sinr": sinr,
        })
    return in_maps


def kernel(inputs, positions, w_in, w_out, _trace=False):
    global _COMPILED
    from concourse.bass_utils import run_bass_kernel_spmd

    if _COMPILED is None:
        _COMPILED = _build()
    nc = _COMPILED

    in_maps = _host_prep(inputs, positions, w_in, w_out)
    res = run_bass_kernel_spmd(
        nc, in_maps, core_ids=list(range(NC_)), trace=_trace
    )
    kernel.last_results = res

    acc = np.zeros((D, GS), np.float32)
    for c in range(NC_):
        acc += np.asarray(res.results[c]["outT"], dtype=np.float32)
    return np.ascontiguousarray(acc.T).reshape(B, S, D)


# revision 46
# speedup vs baseline: 3.6279x; 1.1331x over previous
"""Distributed Trainium2 kernel for the AttentionBlock problem.

Sharding (v2): tensor-parallel over heads for QKV+attention (each of the 8
cores owns 2 heads for both batches), sequence-parallel for the V projection
and the output projection (each core owns one 512-row block of the flattened
(B*S) dimension).  Two small (1 MB) AllToAll collectives glue the layouts
together:

  1. V is projected seq-parallel (wide, efficient matmuls), then AllToAll'd
     so every core holds V for its own 2 heads over all 4096 rows.  This
     collective overlaps the Q/K projection + RoPE.
  2. After attention, normalized head outputs are AllToAll'd so every core
     holds all 1024 head-dims for its own 512 rows, then applies the full
     output projection locally.  Outputs concatenate on the host.

Device notes:
- All matmul inputs are bf16, PSUM accumulates f32.
- Attention scores are computed transposed (k on partitions, q free) so the
  softmax exp feeds the PV matmul directly; the softmax denominator comes
  from a leading all-ones column prepended to each head's V block.
- Score matmuls for the two batches of a head are row-tiled into the PE
  array concurrently (each uses 64 of the 128 contraction rows).
- The exp runs on the scalar engine at [128,1024] per score block; with
  2-deep score PSUM and 4-deep output PSUM this fits exactly in 8 banks.
"""

import numpy as np
import ml_dtypes

BF16 = ml_dtypes.bfloat16
H, HD, D, B, S = 16, 64, 1024, 2, 2048
LS = 512            # seq rows per core for V / out projection
NC_ = 8
GS = B * S          # 4096 flattened rows
ROPE_THETA = 10000.0

_COMPILED = None


def _build(stage=3):
    import concourse.bass as bass
    import concourse.mybir as mybir
    import concourse.tile as tile
    from concourse import bacc

    fp32 = mybir.dt.float32
    bf16 = mybir.dt.bfloat16

    nc = bacc.Bacc(
        "TRN2", target_bir_lowering=False, debug=False, num_devices=NC_
    )

    xT = nc.dram_tensor("xT", [D, GS], bf16, kind="ExternalInput")
    wqkv = nc.dram_tensor("wqkv", [D, 384], bf16, kind="ExternalInput")
    woutl = nc.dram_tensor("woutl", [128, D], bf16, kind="ExternalInput")
    cosr = nc.dram_tensor("cosr", [128, GS], bf16, kind="ExternalInput")
    sinr = nc.dram_tensor("sinr", [128, GS], bf16, kind="ExternalInput")
    # per-core partial of the output projection, transposed: rows = out
    # dims, cols = global (b*S + s) rows; host sums the 8 partials.
    outT = nc.dram_tensor("outT", [D, GS], bf16, kind="ExternalOutput")

    Exp = mybir.ActivationFunctionType.Exp
    Copy = mybir.ActivationFunctionType.Copy
    Recip = mybir.ActivationFunctionType.Reciprocal

    with tile.TileContext(nc) as tc:
        dma = nc.default_dma_engine
        _keep = []

        def _single(*args, **kwargs):
            t, f = tc.tile(*args, **kwargs)
            _keep.append(f)
            return t

        # ---- persistent SBUF tensors ----
        wqkv_sb = _single([128, 8, 384], bf16, name="wqkv_sb")
        ident = _single([128, 128], bf16, name="ident")
        cos_sb = _single([128, GS], bf16, name="cos_sb")
        sin_sb = _single([128, GS], bf16, name="sin_sb")
        wout_sb = _single([128, D], bf16, name="wout_sb")
        o1_all = _single([128, GS], bf16, name="o1_all")
        o2_all = _single([128, GS], bf16, name="o2_all")
        # per-batch head-pair tiles: rows 0:64 = head h0, 64:128 = head h1
        qp = [_single([128, S], bf16, name=f"qp{b}") for b in range(2)]
        khp = [_single([128, S], bf16, name=f"khp{b}") for b in range(2)]
        vo_all = _single([128, 32, 130], bf16, name="vo_all")
        # normalized attention outputs: [my 128 head dims, slot = 4b+j, q]
        o_all = _single([128, 8, LS], bf16, name="o_all")

        # ---- input DMAs, in pipeline order ----
        dma.dma_start(out=wqkv_sb[:],
                      in_=wqkv[:].rearrange("(c p) k -> p c k", p=128))
        xq_pool_cm = tc.tile_pool(name="xq_pool", bufs=2)
        xq_pool = xq_pool_cm.__enter__()
        xq_tiles = []
        for q4 in range(2):
            xq = xq_pool.tile([128, 8, 1024], bf16, tag="xq")
            dma.dma_start(
                out=xq[:],
                in_=xT[:, 1024 * q4:1024 * (q4 + 1)].rearrange(
                    "(c p) s -> p c s", p=128),
            )
            xq_tiles.append(xq)
        dma.dma_start(out=cos_sb[:, 0:S], in_=cosr[:, 0:S])
        dma.dma_start(out=sin_sb[:, 0:S], in_=sinr[:, 0:S])
        dma.dma_start(out=cos_sb[:, S:GS], in_=cosr[:, S:GS])
        dma.dma_start(out=sin_sb[:, S:GS], in_=sinr[:, S:GS])
        dma.dma_start(out=wout_sb[:], in_=woutl[:])

        from concourse import masks as _masks
        _masks.make_identity(nc, ident[:])

        # vo_all[p, kbg, [v_h0(64) | 1 | v_h1(64) | 1]]
        nc.vector.memset(vo_all[:, :, 64:65], 1.0)
        nc.vector.memset(vo_all[:, :, 129:130], 1.0)

        # ========== phase 1: QKV projection + rope + V transpose ==========
        # Per 512-seq chunk: ps1 = qk_x1 channels, ps2 = qk_x2 channels
        # (both roped), ps3 = v^T channels (PE-transposed into vo_all).
        with (
            tc.tile_pool(name="ps_kq", bufs=6, space="PSUM") as ps_kq,
            tc.tile_pool(name="ps_tr", bufs=2, space="PSUM") as ps_tr,
            tc.tile_pool(name="rope_t", bufs=4) as rope_t,
            tc.tile_pool(name="vtp", bufs=2) as vtp,
        ):
            for q4 in range(4):
                if q4 < 2:
                    xq = xq_tiles[q4]
                else:
                    xq = xq_pool.tile([128, 8, 1024], bf16, tag="xq")
                    dma.dma_start(
                        out=xq[:],
                        in_=xT[:, 1024 * q4:1024 * (q4 + 1)].rearrange(
                            "(c p) s -> p c s", p=128),
                    )
                for s2 in range(2):
                    sl = slice(1024 * q4 + 512 * s2, 1024 * q4 + 512 * (s2 + 1))
                    xsl = slice(512 * s2, 512 * (s2 + 1))
                    ps1 = ps_kq.tile([128, 512], fp32, tag="pskq")
                    for d8 in range(8):
                        nc.tensor.matmul(
                            ps1[:], wqkv_sb[:, d8, 0:128], xq[:, d8, xsl],
                            start=(d8 == 0), stop=(d8 == 7),
                        )
                    ps2 = ps_kq.tile([128, 512], fp32, tag="pskq")
                    for d8 in range(8):
                        nc.tensor.matmul(
                            ps2[:], wqkv_sb[:, d8, 128:256], xq[:, d8, xsl],
                            start=(d8 == 0), stop=(d8 == 7),
                        )
                    ps3 = ps_kq.tile([128, 512], fp32, tag="pskq")
                    for d8 in range(8):
                        nc.tensor.matmul(
                            ps3[:], wqkv_sb[:, d8, 256:384], xq[:, d8, xsl],
                            start=(d8 == 0), stop=(d8 == 7),
                        )
                    cs, sn = cos_sb[:, sl], sin_sb[:, sl]
                    t1 = rope_t.tile([128, 512], bf16, tag="rt")
                    t2 = rope_t.tile([128, 512], bf16, tag="rt")
                    nc.vector.tensor_mul(t1[:], ps1[:], cs)
                    nc.vector.tensor_mul(t2[:], ps2[:], sn)
                    nc.vector.tensor_sub(o1_all[:, sl], t1[:], t2[:])
                    nc.vector.tensor_mul(t1[:], ps1[:], sn)
                    nc.vector.tensor_mul(t2[:], ps2[:], cs)
                    nc.vector.tensor_add(o2_all[:, sl], t1[:], t2[:])
                    # v^T -> bf16 -> PE transpose -> vo_all (natural layout)
                    vt = vtp.tile([128, 512], bf16, tag="vt")
                    nc.vector.tensor_copy(vt[:], ps3[:])
                    for t4 in range(4):
                        kbg = 8 * q4 + 4 * s2 + t4
                        ptr = ps_tr.tile([128, 128], bf16, tag="ptr")
                        nc.tensor.transpose(
                            ptr[:], vt[:, 128 * t4:128 * (t4 + 1)], ident[:])
                        nc.vector.tensor_copy(vo_all[:, kbg, 0:64],
                                              ptr[:, 0:64])
                        nc.vector.tensor_copy(vo_all[:, kbg, 65:129],
                                              ptr[:, 64:128])
                if q4 % 2 == 1:
                    # batch bb fully roped: assemble its head-pair tiles
                    # qp/khp rows: 0:32 h0_x1', 32:64 h0_x2', 64:96 h1_x1',
                    # 96:128 h1_x2'
                    bb = q4 // 2
                    bsl = slice(S * bb, S * (bb + 1))
                    for hl in range(2):
                        dma.dma_start(
                            out=qp[bb][64 * hl:64 * hl + 32, :],
                            in_=o1_all[32 * hl:32 * (hl + 1), bsl])
                        dma.dma_start(
                            out=qp[bb][64 * hl + 32:64 * hl + 64, :],
                            in_=o2_all[32 * hl:32 * (hl + 1), bsl])
                        dma.dma_start(
                            out=khp[bb][64 * hl:64 * hl + 32, :],
                            in_=o1_all[64 + 32 * hl:64 + 32 * (hl + 1), bsl])
                        dma.dma_start(
                            out=khp[bb][64 * hl + 32:64 * hl + 64, :],
                            in_=o2_all[64 + 32 * hl:64 + 32 * (hl + 1), bsl])

        xq_pool_cm.__exit__(None, None, None)

        if stage == 1:
            with tc.tile_pool(name="dbg", bufs=2) as dbg:
                o = dbg.tile([128, LS], fp32, name="dbgo")
                nc.vector.tensor_copy(o[:, 0:64], vo_all[:, 0, 0:64])
                nc.vector.tensor_copy(o[:, 64:128], vo_all[:, 17, 65:129])
                nc.vector.memset(o[:, 128:LS], 0.0)
                dma.dma_start(out=outT[0:128, :], in_=o[:])
                o2 = dbg.tile([128, LS], fp32, name="dbgo2")
                nc.vector.tensor_copy(o2[:], qp[0][:, 0:LS])
                dma.dma_start(out=outT[128:256, :], in_=o2[:])
                o3 = dbg.tile([128, LS], fp32, name="dbgo3")
                nc.vector.tensor_copy(o3[:], khp[0][:, 0:LS])
                dma.dma_start(out=outT[256:384, :], in_=o3[:])
            for f in reversed(_keep):
                f()
            nc.compile()
            return nc

        # ================= phase 3: attention ==============================
        with (
            tc.tile_pool(name="ps_sc", bufs=2, space="PSUM") as ps_sc,
            tc.tile_pool(name="ps_out", bufs=3, space="PSUM") as ps_out,
            tc.tile_pool(name="ps_op", bufs=1, space="PSUM") as ps_op,
            tc.tile_pool(name="p_pool", bufs=32) as p_pool,
            tc.tile_pool(name="fin", bufs=4) as fin,
            tc.tile_pool(name="ocp", bufs=4) as ocp,
        ):
            def oproj_step(slot, oc):
                # one 128-col chunk of the partial output projection; its
                # matmuls are spread between attention score/PV matmuls so
                # they never stall the exp stream, and its copies run on DVE.
                ps = ps_op.tile([128, 512], fp32, tag="psop")
                nc.tensor.matmul(
                    ps[:], wout_sb[:, 128 * oc:128 * (oc + 1)],
                    o_all[:, slot, :],
                    start=True, stop=True,
                )
                ot = ocp.tile([128, 512], bf16, tag="ocp")
                nc.vector.tensor_copy(ot[:], ps[:])
                dma.dma_start(
                    out=outT[128 * oc:128 * (oc + 1),
                             512 * slot:512 * (slot + 1)],
                    in_=ot[:])

            def emit_job(bb, j, oslot):
                qsl = slice(512 * j, 512 * (j + 1))
                outp = [ps_out.tile([128, 512], fp32, tag="pso",
                                    name=f"outp{bb}_{j}_{u}")
                        for u in range(2)]
                for kb in range(16):
                    sc_ps = ps_sc.tile([128, 1024], fp32, tag="sc")
                    ksl = slice(128 * kb, 128 * (kb + 1))
                    for u in range(2):
                        nc.tensor.matmul(
                            sc_ps[:, 512 * u:512 * (u + 1)],
                            khp[bb][64 * u:64 * (u + 1), ksl],
                            qp[bb][64 * u:64 * (u + 1), qsl],
                            start=True, stop=True,
                        )
                    p_sb = p_pool.tile([128, 1024], bf16, tag="p")
                    nc.scalar.activation(p_sb[:], sc_ps[:], Exp, scale=0.125)
                    for u in range(2):
                        nc.tensor.matmul(
                            outp[u][0:65, :],
                            vo_all[:, 16 * bb + kb, 65 * u:65 * (u + 1)],
                            p_sb[:, 512 * u:512 * (u + 1)],
                            start=(kb == 0), stop=(kb == 15),
                            skip_group_check=True,
                        )
                    if oslot is not None and kb % 2 == 1:
                        oproj_step(oslot, kb // 2)
                slot = 4 * bb + j
                for u in range(2):
                    dsb = fin.tile([1, 512], fp32, tag="dsb")
                    nc.vector.tensor_copy(dsb[:], outp[u][64:65, :])
                    recip = fin.tile([1, 512], fp32, tag="recip")
                    nc.vector.reciprocal_approx_fast(recip[:], dsb[:])
                    bcast = fin.tile([64, 512], fp32, tag="bcast")
                    nc.gpsimd.partition_broadcast(bcast[:], recip[:])
                    nc.vector.tensor_mul(
                        o_all[64 * u:64 * (u + 1), slot, :],
                        outp[u][0:64, :], bcast[:])

            sched = [(0, 0, None), (0, 1, None), (0, 2, 0), (0, 3, 1),
                     (1, 0, 2), (1, 1, 3), (1, 2, 4), (1, 3, 5)]
            for bb, j, opr in sched:
                emit_job(bb, j, opr)

            # last two slots drain through the (now free) score pool,
            # two output chunks per 2-bank tile.
            for slot in (6, 7):
                for oc2 in range(4):
                    ps = ps_sc.tile([128, 1024], fp32, tag="sc")
                    for half in range(2):
                        oc = 2 * oc2 + half
                        nc.tensor.matmul(
                            ps[:, 512 * half:512 * (half + 1)],
                            wout_sb[:, 128 * oc:128 * (oc + 1)],
                            o_all[:, slot, :],
                            start=True, stop=True,
                        )
                    ot = ocp.tile([128, 1024], bf16, tag="ocp2")
                    nc.vector.tensor_copy(ot[:], ps[:])
                    dma.dma_start(
                        out=outT[256 * oc2:256 * (oc2 + 1),
                                 512 * slot:512 * (slot + 1)].rearrange(
                                     "(two p) q -> p two q", two=2),
                        in_=ot[:])

        for f in reversed(_keep):
            f()

    nc.compile()
    return nc


def _host_prep(inputs, positions, w_in, w_out):
    inputs = np.asarray(inputs, np.float32)
    positions = np.asarray(positions)
    w_in = np.asarray(w_in, np.float32)
    w_out = np.asarray(w_out, np.float32)

    x_all = np.concatenate([inputs[0], inputs[1]], axis=0)          # (4096, D)
    xT_full = np.ascontiguousarray(x_all.T).astype(BF16)            # (D, 4096)

    ar32, ar64 = np.arange(32), np.arange(64)

    inv_freq = 1.0 / (ROPE_THETA ** (np.arange(32, dtype=np.float32) / 32))
    pos_all = np.concatenate([positions[0], positions[1]]).astype(np.float32)
    ang = pos_all[None, :] * inv_freq[:, None]                      # (32, 4096)
    cosr = np.ascontiguousarray(np.tile(np.cos(ang), (4, 1))).astype(BF16)
    sinr = np.ascontiguousarray(np.tile(np.sin(ang), (4, 1))).astype(BF16)

    in_maps = []
    for c in range(NC_):
        H0, H1 = 2 * c, 2 * c + 1
        cols = np.concatenate([
            192 * H0 + ar32, 192 * H1 + ar32,            # q_x1 h0, h1
            192 * H0 + 64 + ar32, 192 * H1 + 64 + ar32,  # k_x1 h0, h1
            192 * H0 + 32 + ar32, 192 * H1 + 32 + ar32,  # q_x2 h0, h1
            192 * H0 + 96 + ar32, 192 * H1 + 96 + ar32,  # k_x2 h0, h1
            192 * H0 + 128 + ar64, 192 * H1 + 128 + ar64,  # v h0, h1
        ])
        wqkv = np.ascontiguousarray(w_in[:, cols]).astype(BF16)
        # rows of w_out for my two heads' output dims
        woutl = np.ascontiguousarray(
            w_out[128 * c:128 * (c + 1), :]).astype(BF16)
        in_maps.append({
            "xT": xT_full, "wqkv": wqkv,
            "woutl": woutl, "cosr": cosr, "sinr": sinr,
        })
    return in_maps


def kernel(inputs, positions, w_in, w_out, _trace=False):
    global _COMPILED
    from concourse.bass_utils import run_bass_kernel_spmd

    if _COMPILED is None:
        _COMPILED = _build()
    nc = _COMPILED

    in_maps = _host_prep(inputs, positions, w_in, w_out)
    res = run_bass_kernel_spmd(
        nc, in_maps, core_ids=list(range(NC_)), trace=_trace
    )
    kernel.last_results = res

    acc = np.zeros((D, GS), np.float32)
    for c in range(NC_):
        acc += np.asarray(res.results[c]["outT"], dtype=np.float32)
    return np.ascontiguousarray(acc.T).reshape(B, S, D)
